# revision 55
# baseline (speedup 1.0000x reference)
"""Trainium2 Bass kernel for nn_BasicLayer (Erwin NSA-MSA sparse ball attention).

8 NeuronCores, data-parallel over points: each core receives the full x/pos
ROTATED so its own 1024 points sit at rows [0:1024] (whole-ball rotation makes
one SPMD program correct for every core; no collectives).

Per core:
  - Stage A: xr = rmsnorm(x)*n1w + rel for all 8192 points, ball-per-partition;
    bf16 copy of xr kept as the gather source; ball-summary keys by reduction.
    Ball pos-means via flat halving folds; rsqrt = exp(-0.5*ln(v)) + Newton
    (keeps ACT in the natural_log_exp table set used by attention exp).
  - Routing logits via 4-term bf16-split PE matmuls (bit-exact vs fp32 ref, so
    top-2 ball selection matches jax.lax.top_k except true fp32 ties).
  - Per (head, tile): DVE max8 -> equality masks [n,b] in bf16 -> PE transpose
    -> PE one-hot gather matmuls -> single PSUM->SBUF bf16 evac (ACT).
  - Per head, batched across tiles (DVE tensor_reduce runs at 1x, so all
    reductions are pairwise halving folds that hit the 2x bf16/fp16 TT mode):
    scores product (2x: e-innermost broadcast AP packs) + 3 e-pair-folds;
    exp emitted e-REPLICATED by ACT (step-0 input re-read) so the
    weighted-sum product is a flat 2x TT, in tile-halves for ACT/DVE overlap;
    z by strided reduce of one e-lane; (k,m)-reduce via contiguous [m,e]-run
    halving folds in fp16.
  - Residual + RMSNorm + SwiGLU MLP (fp16 weights/transposes/matmuls on PE --
    fp32 PE runs at quarter rate; Silu in one ACT op, emitted last so its
    table set loads once).
"""

import numpy as np

import concourse.bacc as bacc
import concourse.bass as bass
import concourse.mybir as mybir
import concourse.tile as tile
from concourse.masks import make_identity

FP = mybir.dt.float32
BF = mybir.dt.bfloat16
F16 = mybir.dt.float16
U16 = mybir.dt.uint16
I16 = mybir.dt.int16

N, D = 8192, 64
M = 64          # ball size
NB = N // M     # 128 balls
H, EH = 8, 8
TOPK = 2
NCORES = 8
NPC = N // NCORES   # 1024 points per core
NT = NPC // 128     # 8 point-tiles of 128
BPC = NPC // M      # 16 own balls per core
DH = D * 4          # 256 mlp hidden
EPS = 1.1920929e-07
ISQ8 = float(1.0 / np.sqrt(EH))
EM = M * EH         # 512 = gathered elem size (m-major, e innermost)
NG = NT * TOPK      # 16 gather slots per point
NIDX = NG * 128     # 2048 gathered blocks per head

A = mybir.AluOpType
AF = mybir.ActivationFunctionType
AX = mybir.AxisListType


def _bc(ap, dim, count):
    """Insert a step-0 (broadcast) dim at position `dim` of an AP."""
    new = [list(p) for p in ap.ap]
    new.insert(dim, [0, count])
    return bass.AP(tensor=ap.tensor, offset=ap.offset, ap=new)


def build_kernel_body(nc, tc, ctx, tensors):
    (x_d, pos_d, n1w_d, n2w_d, w1_d, b1_d, w2_d, b2_d, w3_d, b3_d,
     out_d, xr_dram) = tensors

    consts = ctx.enter_context(tc.tile_pool(name="consts", bufs=1))
    big = ctx.enter_context(tc.tile_pool(name="big", bufs=1))
    front_cm = tc.tile_pool(name="front", bufs=1)
    front = front_cm.__enter__()
    ps_tr_cm = tc.tile_pool(name="ps_tr", bufs=2, space="PSUM")
    ps_tr = ps_tr_cm.__enter__()

    ident = consts.tile([128, 128], FP)
    make_identity(nc, ident)


    # ---------------- Stage A: load + xr = rmsnorm(x)*n1w + rel (ball-major) ----
    x_bm = front.tile([128, M, D], FP)       # [ball, m, d]
    pos_bm = front.tile([128, M, D], FP)
    # x and pos on different DMA queues so the two 2MB loads overlap
    nc.sync.dma_start(out=x_bm, in_=x_d.ap().rearrange("(b m) d -> b m d", m=M))
    nc.gpsimd.dma_start(out=pos_bm,
                        in_=pos_d.ap().rearrange("(b m) d -> b m d", m=M))

    n1w_sb = consts.tile([128, D], FP)
    nc.sync.dma_start(out=n1w_sb,
                      in_=bass.AP(tensor=n1w_d, offset=0, ap=[[0, 128], [1, D]]))

    # ball mean of pos (over m): flat contiguous halving folds (m-major)
    mpf = front.tile([128, 32, D], FP, tag="mpf")
    nc.vector.tensor_tensor(out=mpf, in0=pos_bm[:, 0:32, :],
                            in1=pos_bm[:, 32:64, :], op=A.add)
    w = 16
    while w >= 1:
        nc.vector.tensor_tensor(out=mpf[:, 0:w, :], in0=mpf[:, 0:w, :],
                                in1=mpf[:, w:2 * w, :], op=A.add)
        w //= 2
    mp = front.tile([128, D], FP, tag="mp")
    nc.vector.tensor_scalar(mp, mpf[:, 0, :], 1.0 / M, None, op0=A.mult)

    # rms: 1/sqrt(mean(x^2) + eps)
    sq = front.tile([128, M, D], FP, tag="sq")
    nc.scalar.activation(out=sq, in_=x_bm, func=AF.Square)
    sq8 = front.tile([128, M, 8], FP, tag="sq8")
    nc.vector.tensor_reduce(out=sq8, in_=sq.rearrange("b m (g d) -> b m g d", g=8),
                            axis=AX.X, op=A.add)
    msq = front.tile([128, M], FP, tag="msq")
    nc.vector.tensor_reduce(out=msq, in_=sq8, axis=AX.X, op=A.add)
    nc.vector.tensor_scalar(msq, msq, 1.0 / D, EPS, op0=A.mult, op1=A.add)
    rinv = front.tile([128, M], FP, tag="rinv")
    lnv = front.tile([128, M], FP, tag="lnv")
    nc.scalar.activation(out=lnv, in_=msq, func=AF.Ln)
    nc.scalar.activation(out=rinv, in_=lnv, func=AF.Exp, scale=-0.5)
    # one Newton step: r <- r*(1.5 - 0.5*msq*r^2)
    rsqv = front.tile([128, M], FP, tag="rsqv")
    nc.vector.tensor_tensor(out=rsqv, in0=rinv, in1=rinv, op=A.mult)
    nc.vector.tensor_tensor(out=rsqv, in0=rsqv, in1=msq, op=A.mult)
    nc.vector.tensor_scalar(rsqv, rsqv, -0.5, 1.5, op0=A.mult, op1=A.add)
    nc.vector.tensor_tensor(out=rinv, in0=rinv, in1=rsqv, op=A.mult)

    nc.vector.tensor_tensor(out=pos_bm, in0=pos_bm, in1=_bc(mp, 1, M),
                            op=A.subtract)      # pos_bm becomes rel
    xr_bm = front.tile([128, M, D], FP)
    nc.vector.tensor_tensor(out=xr_bm, in0=x_bm, in1=_bc(rinv, 2, D), op=A.mult)
    nc.vector.tensor_tensor(out=xr_bm, in0=xr_bm, in1=_bc(n1w_sb, 1, M), op=A.mult)
    nc.vector.tensor_tensor(out=xr_bm, in0=xr_bm, in1=pos_bm, op=A.add)

    # ---------------- Stage B: KB to DRAM (bf16) + ball-summary keys ----------
    xr16 = big.tile([128, M, D], BF)
    nc.vector.tensor_copy(out=xr16, in_=xr_bm)

    # ball-summary keys: flat halving folds over m (strided reduce is 2x slower)
    kf = front.tile([128, 32, D], FP, tag="kf")
    nc.vector.tensor_tensor(out=kf, in0=xr_bm[:, 0:32, :],
                            in1=xr_bm[:, 32:64, :], op=A.add)
    w = 16
    while w >= 1:
        nc.vector.tensor_tensor(out=kf[:, 0:w, :], in0=kf[:, 0:w, :],
                                in1=kf[:, w:2 * w, :], op=A.add)
        w //= 2
    keys_bm = kf[:, 0, :]
    keysT = front.tile([64, 128], FP)                 # [(h e), ball]
    kt_ps = ps_tr.tile([64, 128], FP, tag="tr")
    nc.tensor.transpose(kt_ps, keys_bm, ident)
    nc.scalar.copy(out=keysT, in_=kt_ps)

    # ---------------- Stage C: own-point layouts --------------------------------
    # xr rows [0:1024] -> DRAM bounce -> point-major + transposed copies
    nc.sync.dma_start(out=xr_dram.ap().rearrange("(b m) d -> b m d", m=M),
                      in_=xr_bm[0:BPC, :, :])
    q_pm = big.tile([128, NT, H, EH], FP)  # per-partition q scalars
    nc.sync.dma_start(out=q_pm,
                      in_=xr_dram.ap().rearrange("(t p) (h e) -> p t h e", p=128, e=EH))
    x_own = big.tile([128, NT, D], FP)
    nc.sync.dma_start(out=x_own,
                      in_=x_d.ap()[0:NPC, :].rearrange("(t p) d -> p t d", p=128))

    qT = front.tile([64, NT, 128], FP)       # [(h e), nt, n128]
    for t in range(NT):
        q_ps = ps_tr.tile([64, 128], FP, tag="tr")
        nc.tensor.transpose(q_ps, q_pm[:, t].rearrange("p h e -> p (h e)"), ident)
        nc.scalar.copy(out=qT[:, t, :], in_=q_ps)

    # hi/lo bf16 split: 4-term split q.k = qhi.khi + qhi.klo + qlo.khi + qlo.klo
    # -> every bf16 product exact in fp32 -> logits match fp32 ref to ~2 ulp.
    # K-row order p = e*4 + j ; k terms [khi, klo, khi, klo], q [qhi, qhi, qlo, qlo]
    identB = consts.tile([128, 128], BF)
    nc.vector.tensor_copy(out=identB, in_=ident)
    kst4 = front.tile([64, 4, 128], BF)
    qst4 = front.tile([64, 4, NT, 128], BF)
    tmp32 = front.tile([64, NT, 128], FP, tag="tmp32")
    for (src_ap, dst, nfree) in ((keysT, kst4, 1), (qT, qst4, NT)):
        nc.vector.tensor_copy(out=dst[:, 0], in_=src_ap)     # hi (cast bf16)
        t32 = tmp32[:, 0:nfree, :] if nfree == NT else tmp32[:, 0, :]
        nc.vector.tensor_copy(out=t32, in_=dst[:, 0])        # hi back to fp32
        nc.vector.tensor_tensor(out=t32, in0=src_ap, in1=t32, op=A.subtract)
        nc.vector.tensor_copy(out=dst[:, 1], in_=t32)        # lo (cast bf16)
        if nfree == 1:   # k: [khi, klo, khi, klo]
            nc.vector.tensor_copy(out=dst[:, 2], in_=dst[:, 0])
            nc.vector.tensor_copy(out=dst[:, 3], in_=dst[:, 1])
        else:            # q: [qhi, qhi, qlo, qlo]
            nc.vector.tensor_copy(out=dst[:, 2], in_=dst[:, 1])
            nc.vector.tensor_copy(out=dst[:, 3], in_=dst[:, 1])
            nc.vector.tensor_copy(out=dst[:, 1], in_=dst[:, 0])
    kstack = big.tile([32, H, 128], BF)
    qstack = big.tile([32, H, NT, 128], BF)
    for h in range(H):
        # two DMA queues so the 16 strided loads overlap (head order kept)
        nc.sync.dma_start(
            out=kstack[:, h, :],
            in_=bass.AP(tensor=kst4.tensor, offset=kst4.offset + 8 * h * 4 * 128,
                        ap=[[4 * 128, 8], [128, 4], [1, 128]]))
        nc.gpsimd.dma_start(
            out=qstack[:, h],
            in_=bass.AP(tensor=qst4.tensor,
                        offset=qst4.offset + 8 * h * 4 * NT * 128,
                        ap=[[4 * NT * 128, 8], [NT * 128, 4], [128, NT],
                            [1, 128]]))
    # weights: w1/w2 [256, 64] -> transposed fp16 [64, 256]; w3 [64, 256] ->
    # [256, 64] (fp16: fp32 PE matmuls/transposes run at quarter rate)
    w1n = consts.tile([128, 2, D], FP)
    w2n = consts.tile([128, 2, D], FP)
    w3n = consts.tile([64, DH], FP)
    nc.sync.dma_start(out=w1n, in_=w1_d.ap().rearrange("(c j) d -> j c d", j=128))
    nc.sync.dma_start(out=w2n, in_=w2_d.ap().rearrange("(c j) d -> j c d", j=128))
    nc.sync.dma_start(out=w3n, in_=w3_d.ap())
    w1h = consts.tile([128, 2, D], F16)
    w2h = consts.tile([128, 2, D], F16)
    w3h = consts.tile([64, DH], F16)
    nc.vector.tensor_copy(out=w1h, in_=w1n)
    nc.vector.tensor_copy(out=w2h, in_=w2n)
    nc.vector.tensor_copy(out=w3h, in_=w3n)
    identH = consts.tile([128, 128], F16)
    nc.vector.tensor_copy(out=identH, in_=ident)
    w1T = consts.tile([64, DH], F16)   # [d, j]
    w2T = consts.tile([64, DH], F16)
    w3T = consts.tile([128, 2, D], F16)  # [j128, c, e]
    for c in range(2):
        t_ps = ps_tr.tile([64, 128], F16, tag="trh")
        nc.tensor.transpose(t_ps, w1h[:, c, :], identH)
        nc.scalar.copy(out=w1T[:, 128 * c:128 * (c + 1)], in_=t_ps)
        t_ps2 = ps_tr.tile([64, 128], F16, tag="trh")
        nc.tensor.transpose(t_ps2, w2h[:, c, :], identH)
        nc.scalar.copy(out=w2T[:, 128 * c:128 * (c + 1)], in_=t_ps2)
        t_ps3 = ps_tr.tile([128, 64], F16, tag="trh2")
        nc.tensor.transpose(t_ps3, w3h[:, 128 * c:128 * (c + 1)],
                            identH[0:64, 0:64])
        nc.scalar.copy(out=w3T[:, c, :], in_=t_ps3)
    b1_sb = consts.tile([128, 2], FP)
    b2_sb = consts.tile([128, 2], FP)
    nc.sync.dma_start(out=b1_sb, in_=b1_d.ap().rearrange("(c j) -> j c", j=128))
    nc.sync.dma_start(out=b2_sb, in_=b2_d.ap().rearrange("(c j) -> j c", j=128))
    b3_sb = consts.tile([128, D], FP)
    nc.sync.dma_start(out=b3_sb,
                      in_=bass.AP(tensor=b3_d, offset=0, ap=[[0, 128], [1, D]]))

    front_cm.__exit__(None, None, None)
    ps_tr_cm.__exit__(None, None, None)

    # ---------------- Stage D+E: selection + PE one-hot gather + attention -----
    # Per (h, t): PE logits -> DVE max8 -> DVE equality masks [n, b] (bf16)
    # -> PE transpose -> evac maskT -> PE gather matmuls -> evac G to bf16.
    # Then per-head batched DVE scores/softmax/weighted-sum (as v2).
    gpool = ctx.enter_context(tc.tile_pool(name="gpool", bufs=4))
    wpool = ctx.enter_context(tc.tile_pool(name="wpool", bufs=2))
    sel_cm = tc.tile_pool(name="sel", bufs=8)
    sel = sel_cm.__enter__()
    ps_lt_cm = tc.tile_pool(name="ps_lt", bufs=2, space="PSUM")
    ps_lt = ps_lt_cm.__enter__()
    ps_mt_cm = tc.tile_pool(name="ps_mt", bufs=2, space="PSUM")
    ps_mt = ps_mt_cm.__enter__()
    ps_g_cm = tc.tile_pool(name="ps_g", bufs=2, space="PSUM")
    ps_g = ps_g_cm.__enter__()

    attn16 = big.tile([128, NT, H, EH], BF)
    zh = big.tile([128, H, NT], FP)

    def selection(h, ts):
        """Logits (PE) + top-2 masks (DVE) for head h, tiles ts; returns masks.

        Split from the gather so its DVE ops can be emitted while head h-1's
        ACT exp runs, and the gather's ACT ops (maskT + evac) can be emitted
        AFTER head h-1's exp — engines are strict FIFO, so emission order on
        each engine is execution order."""
        masks = {}
        for t in ts:
            lpm_ps = ps_lt.tile([128, 128], FP, tag="lt")
            nc.tensor.matmul(lpm_ps, lhsT=qstack[:, h, t, :],
                             rhs=kstack[:, h, :], start=True, stop=True)
            # evac logits to SBUF on ACT: PSUM operands cap DVE ops at 1x
            # with a +62-cycle access penalty; fp32 copy is exact
            lpm_sb = sel.tile([128, 128], FP, tag="lpm")
            nc.scalar.copy(out=lpm_sb, in_=lpm_ps)
            v8t = sel.tile([128, 8], FP, tag="v8")
            nc.vector.max(out=v8t, in_=lpm_sb)
            # both tk masks in one op: mask2[p, tk, b] = (lpm == v8[tk])
            mask2 = sel.tile([128, TOPK, 128], BF, tag="mnb")
            nc.vector.tensor_tensor(
                out=mask2, in0=_bc(lpm_sb, 1, TOPK),
                in1=bass.AP(tensor=v8t.tensor, offset=v8t.offset,
                            ap=[v8t.ap[0], [1, TOPK], [0, 128]]),
                op=A.is_equal)
            masks[t] = mask2
        return masks

    def gather_head(h, masks, ts, g_sb=None):
        """PE transposes + one-hot gathers + ACT evacs for head h, tiles ts.

        maskT(t+1) is emitted BEFORE evac(t) so the strict-FIFO ACT queue
        doesn't serialize ACT->PE->ACT per tile: gather(t) runs on PE while
        ACT does maskT(t+1), and evac(t) follows immediately."""
        if g_sb is None:
            g_sb = gpool.tile([128, NT, TOPK, M, EH], BF, tag="g")

        def emit_maskT_gather(t):
            mask2 = masks[t]
            mt_ps = ps_mt.tile([128, TOPK, 128], BF, tag="mt")
            for tk in range(TOPK):
                nc.tensor.transpose(mt_ps[:, tk, :], mask2[:, tk, :], identB)
            maskT = sel.tile([128, TOPK, 128], BF, tag="mT")
            nc.scalar.copy(out=maskT, in_=mt_ps)
            g_ps = ps_g.tile([128, TOPK, M, EH], FP, tag="g")
            for tk in range(TOPK):
                # rhs: xr16 [b, m, (h e)] -> per-head (m, e) column order
                nc.tensor.matmul(
                    g_ps[:, tk].rearrange("p m e -> p (m e)"),
                    lhsT=maskT[:, tk, :],
                    rhs=bass.AP(tensor=xr16.tensor,
                                offset=xr16.offset + EH * h,
                                ap=[xr16.ap[0], [D, M], [1, EH]]),
                    start=True, stop=True)
            return g_ps

        gp_prev, t_prev = emit_maskT_gather(ts[0]), ts[0]
        for t in ts[1:]:
            gp = emit_maskT_gather(t)
            # evac PSUM -> bf16 SBUF on ACT (DVE is the bottleneck engine)
            nc.scalar.copy(out=g_sb[:, t_prev], in_=gp_prev)
            gp_prev, t_prev = gp, t
        nc.scalar.copy(out=g_sb[:, t_prev], in_=gp_prev)
        return g_sb

    def compute_a(h, g_sb):
        """Scores + batched exp for head h (DVE prod/e-folds, ACT exp)."""
        nt = NT
        ng = nt * TOPK
        # bf16 q for this head, replicated over topk: [p, (t k), e]
        q2h = wpool.tile([128, nt, TOPK, EH], BF, tag="q2")
        nc.vector.tensor_copy(
            out=q2h,
            in_=bass.AP(tensor=q_pm.tensor,
                        offset=q_pm.offset + EH * h,
                        ap=[q_pm.ap[0], [H * EH, nt], [0, TOPK], [1, EH]]))
        g_v = g_sb.rearrange("p t k m e -> p (t k) m e")
        prod = wpool.tile([128, ng, M, EH], BF, tag="prod")
        q2_bc = bass.AP(tensor=q2h.tensor, offset=q2h.offset,
                        ap=[q2h.ap[0], [EH, ng], [0, M], [1, EH]])
        nc.vector.tensor_tensor(out=prod, in0=g_v, in1=q2_bc, op=A.mult)
        # s[p, (g m)] fp16 = sum_e prod, via 3 pair-folds over e (e pairs are
        # step-1 runs of 4/2/1 -> cheaper than the 1x-mode strided reduce).
        # scr16 is shared by s4 (e-folds) and p2k (m-folds) — disjoint lifetimes
        scr16 = wpool.tile([128, NT * M * EH], F16, tag="scr")
        s4 = scr16[:, 0:ng * M * 4].rearrange("p (j f) -> p j f", f=4)
        nc.vector.tensor_tensor(
            out=s4,
            in0=bass.AP(tensor=prod.tensor, offset=prod.offset,
                        ap=[prod.ap[0], [EH, ng * M], [1, 4]]),
            in1=bass.AP(tensor=prod.tensor, offset=prod.offset + 4,
                        ap=[prod.ap[0], [EH, ng * M], [1, 4]]),
            op=A.add)
        nc.vector.tensor_tensor(
            out=bass.AP(tensor=s4.tensor, offset=s4.offset,
                        ap=[s4.ap[0], [4, ng * M], [1, 2]]),
            in0=bass.AP(tensor=s4.tensor, offset=s4.offset,
                        ap=[s4.ap[0], [4, ng * M], [1, 2]]),
            in1=bass.AP(tensor=s4.tensor, offset=s4.offset + 2,
                        ap=[s4.ap[0], [4, ng * M], [1, 2]]),
            op=A.add)
        s_sb = wpool.tile([128, ng, M], F16, tag="s")
        nc.vector.tensor_tensor(
            out=s_sb,
            in0=bass.AP(tensor=s4.tensor, offset=s4.offset,
                        ap=[s4.ap[0], [4, ng * M]]),
            in1=bass.AP(tensor=s4.tensor, offset=s4.offset + 1,
                        ap=[s4.ap[0], [4, ng * M]]),
            op=A.add)
        # softmax numerator: p = exp(s/sqrt8), written REPLICATED over e by
        # re-reading s 8x (step-0 input dim) -> prod2 becomes a flat 2x TT.
        # Emitted in tile-halves so ACT exp overlaps DVE prod2 in compute_b.
        p_rep = wpool.tile([128, ng, M, EH], BF, tag="prep")
        hj = ng * M // 2
        for hf in range(2):
            sl = slice(hf * hj, (hf + 1) * hj)
            nc.scalar.activation(
                out=p_rep.rearrange("p g m e -> p (g m) e")[:, sl],
                in_=bass.AP(tensor=s_sb.tensor, offset=s_sb.offset + hf * hj,
                            ap=[s_sb.ap[0], [1, hj], [0, EH]]),
                func=AF.Exp, scale=ISQ8)
        return g_v, p_rep, scr16

    def compute_b(h, state, t0=0, t1=NT):
        """Weighted sum + z + (k,m)-folds for head h, tiles [t0:t1) (DVE)."""
        g_v, p_rep, scr16 = state
        nt = t1 - t0
        ng = nt * TOPK
        j0 = t0 * TOPK * M                 # (g m)-flat element base
        base = j0 * EH                     # fully-flat element base
        prod2 = wpool.tile([128, NT * TOPK, M, EH], BF, tag="prod")
        hj = ng * M // 2
        for hf in range(2):
            sl = slice(j0 + hf * hj, j0 + (hf + 1) * hj)
            nc.vector.tensor_tensor(
                out=prod2.rearrange("p g m e -> p (g m) e")[:, sl],
                in0=g_v.rearrange("p g m e -> p (g m) e")[:, sl],
                in1=p_rep.rearrange("p g m e -> p (g m) e")[:, sl],
                op=A.mult)
        # z[p, t] = sum over (tk, m) of p (read one e-lane of the replica)
        nc.vector.tensor_reduce(
            out=zh[:, h, t0:t1],
            in_=bass.AP(tensor=p_rep.tensor, offset=p_rep.offset + base,
                        ap=[p_rep.ap[0], [TOPK * M * EH, nt], [EH, TOPK * M]]),
            axis=AX.X, op=A.add)
        # fold the two topk slots with a packed TT-add, then reduce over m by
        # halving folds on contiguous [m, e] runs (2x-eligible, vs 1x reduce)
        p2k = scr16[:, t0 * M * EH:t1 * M * EH].rearrange(
            "p (t m e) -> p t m e", m=M, e=EH)
        nc.vector.tensor_tensor(
            out=p2k,
            in0=bass.AP(tensor=prod2.tensor, offset=prod2.offset + base,
                        ap=[prod2.ap[0], [TOPK * M * EH, nt], [EH, M], [1, EH]]),
            in1=bass.AP(tensor=prod2.tensor,
                        offset=prod2.offset + base + M * EH,
                        ap=[prod2.ap[0], [TOPK * M * EH, nt], [EH, M], [1, EH]]),
            op=A.add)
        w = (M // 2) * EH
        while w > EH:
            nc.vector.tensor_tensor(
                out=bass.AP(tensor=p2k.tensor, offset=p2k.offset,
                            ap=[p2k.ap[0], [M * EH, nt], [1, w]]),
                in0=bass.AP(tensor=p2k.tensor, offset=p2k.offset,
                            ap=[p2k.ap[0], [M * EH, nt], [1, w]]),
                in1=bass.AP(tensor=p2k.tensor, offset=p2k.offset + w,
                            ap=[p2k.ap[0], [M * EH, nt], [1, w]]),
                op=A.add)
            w //= 2
        nc.vector.tensor_tensor(
            out=attn16[:, t0:t1, h, :],
            in0=bass.AP(tensor=p2k.tensor, offset=p2k.offset,
                        ap=[p2k.ap[0], [M * EH, nt], [1, EH]]),
            in1=bass.AP(tensor=p2k.tensor, offset=p2k.offset + EH,
                        ap=[p2k.ap[0], [M * EH, nt], [1, EH]]),
            op=A.add)

    # finalize tiles live in `big` so the per-half epilogue can interleave
    # with head 7's split compute (attention pools still open)
    zinv = big.tile([128, H, NT], FP)
    attn = big.tile([128, NT, D], FP)
    y = big.tile([128, NT, D], FP)
    sq2 = big.tile([128, NT, D], FP)
    ri2 = big.tile([128, NT], FP)
    ln2 = big.tile([128, NT], FP)
    x2 = big.tile([128, NT, D], F16)
    n2w_sb = consts.tile([128, D], FP)
    nc.sync.dma_start(out=n2w_sb,
                      in_=bass.AP(tensor=n2w_d, offset=0, ap=[[0, 128], [1, D]]))

    def finalize_a(t0, t1):
        """attn normalize + residual + rmsnorm2 + x2 for tiles [t0:t1)."""
        nt = t1 - t0
        nc.vector.reciprocal(out=zinv[:, :, t0:t1], in_=zh[:, :, t0:t1])
        zinv_bc = bass.AP(tensor=zinv.tensor, offset=zinv.offset + t0,
                          ap=[zinv.ap[0], [1, nt], [NT, H], [0, EH]])
        nc.vector.tensor_tensor(
            out=attn.rearrange("p t (h e) -> p t h e", e=EH)[:, t0:t1],
            in0=attn16[:, t0:t1], in1=zinv_bc, op=A.mult)
        nc.vector.tensor_tensor(out=y[:, t0:t1], in0=x_own[:, t0:t1],
                                in1=attn[:, t0:t1], op=A.add)
        nc.scalar.activation(out=sq2[:, t0:t1], in_=y[:, t0:t1], func=AF.Square)
        nc.vector.tensor_reduce(out=ri2[:, t0:t1], in_=sq2[:, t0:t1],
                                axis=AX.X, op=A.add)
        nc.vector.tensor_scalar(ri2[:, t0:t1], ri2[:, t0:t1], 1.0 / D, EPS,
                                op0=A.mult, op1=A.add)
        nc.scalar.activation(out=ln2[:, t0:t1], in_=ri2[:, t0:t1], func=AF.Ln)
        nc.scalar.activation(out=ri2[:, t0:t1], in_=ln2[:, t0:t1],
                             func=AF.Exp, scale=-0.5)
        nc.vector.tensor_tensor(out=x2[:, t0:t1], in0=y[:, t0:t1],
                                in1=_bc(ri2[:, t0:t1], 2, D), op=A.mult)
        nc.vector.tensor_tensor(out=x2[:, t0:t1], in0=x2[:, t0:t1],
                                in1=_bc(n2w_sb, 1, nt), op=A.mult)

    # Software pipeline over heads. Emission order IS execution order on each
    # strict-FIFO engine, so head h+1's selection/gather is emitted in two
    # tile-halves AROUND head h's exp: ACT runs [mT/evac(h+1) 0-3, exp(h),
    # mT/evac(h+1) 4-7] while DVE runs [masks(h+1) 0-3, prod(h), e-folds(h),
    # masks(h+1) 4-7, prod2(h), folds(h)] with no exp-wait bubble.
    half0, half1 = list(range(NT // 2)), list(range(NT // 2, NT))
    g_cur = gather_head(0, selection(0, half0 + half1), half0 + half1)
    for h in range(H - 1):
        m0 = selection(h + 1, half0)
        g_next = gather_head(h + 1, m0, half0)
        st = compute_a(h, g_cur)
        m1 = selection(h + 1, half1)
        gather_head(h + 1, m1, half1, g_sb=g_next)
        compute_b(h, st)
        g_cur = g_next
    st_last = compute_a(H - 1, g_cur)
    ps_g_cm.__exit__(None, None, None)
    ps_mt_cm.__exit__(None, None, None)
    ps_lt_cm.__exit__(None, None, None)
    sel_cm.__exit__(None, None, None)

    mlpw = ctx.enter_context(tc.tile_pool(name="mlpw", bufs=1))
    ps_m = ctx.enter_context(tc.tile_pool(name="ps_m", bufs=1, space="PSUM"))
    x2T = mlpw.tile([64, NT, 128], F16)
    hhT = mlpw.tile([128, 2, NT, 128], F16)   # [j128, c, nt, n]
    final = mlpw.tile([128, NT, D], FP)

    def mlp_half(t0, t1):
        """x2T transposes + SwiGLU MLP (fp16) + residual + out DMA for [t0:t1)."""
        nt = t1 - t0
        for t in range(t0, t1):
            xt_ps = ps_m.tile([64, 128], F16, tag="tr")
            nc.tensor.transpose(xt_ps, x2[:, t, :], identH)
            nc.scalar.copy(out=x2T[:, t, :], in_=xt_ps)
        nc.vector.tensor_tensor(out=y[:, t0:t1], in0=y[:, t0:t1],
                                in1=_bc(b3_sb, 1, nt), op=A.add)
        for c in range(2):
            h1_ps = ps_m.tile([128, nt, 128], FP, tag="h1")
            h2_ps = ps_m.tile([128, nt, 128], FP, tag="h2")
            nc.tensor.matmul(h1_ps.rearrange("j t n -> j (t n)"),
                             lhsT=w1T[:, 128 * c:128 * (c + 1)],
                             rhs=x2T[:, t0:t1].rearrange("d t n -> d (t n)"),
                             start=True, stop=True)
            nc.tensor.matmul(h2_ps.rearrange("j t n -> j (t n)"),
                             lhsT=w2T[:, 128 * c:128 * (c + 1)],
                             rhs=x2T[:, t0:t1].rearrange("d t n -> d (t n)"),
                             start=True, stop=True)
            h1b = mlpw.tile([128, nt, 128], F16, tag="h1b")
            nc.vector.tensor_scalar(h1b, h1_ps, b1_sb[:, c:c + 1], None,
                                    op0=A.add)
            sgm = mlpw.tile([128, nt, 128], F16, tag="sgm")
            nc.scalar.activation(out=sgm, in_=h1b, func=AF.Silu)
            h2s = mlpw.tile([128, nt, 128], F16, tag="h2s")
            nc.vector.tensor_scalar(h2s, h2_ps, b2_sb[:, c:c + 1], None,
                                    op0=A.add)
            nc.vector.tensor_tensor(out=hhT[:, c, t0:t1], in0=sgm, in1=h2s,
                                    op=A.mult)
        for t in range(t0, t1):
            o_ps = ps_m.tile([128, D], FP, tag="o")
            for c in range(2):
                nc.tensor.matmul(o_ps, lhsT=hhT[:, c, t, :], rhs=w3T[:, c, :],
                                 start=(c == 0), stop=(c == 1))
            nc.vector.scalar_tensor_tensor(out=final[:, t], in0=o_ps,
                                           scalar=1.0, in1=y[:, t],
                                           op0=A.mult, op1=A.add)
        nc.sync.dma_start(
            out=out_d.ap().rearrange("(t p) d -> p t d", p=128)[:, t0:t1],
            in_=final[:, t0:t1])

    # Split epilogue: head 7's second-half folds overlap finalize of the
    # first half; the MLP halves pipeline behind finalize.
    compute_b(H - 1, st_last, 0, NT // 2)
    finalize_a(0, NT // 2)
    compute_b(H - 1, st_last, NT // 2, NT)
    finalize_a(NT // 2, NT)
    mlp_half(0, NT // 2)
    mlp_half(NT // 2, NT)


def build_program():
    from contextlib import ExitStack
    nc = bacc.Bacc("TRN2", target_bir_lowering=False, debug=False,
                   num_devices=NCORES)
    x_d = nc.dram_tensor("x", [N, D], FP, kind="ExternalInput")
    pos_d = nc.dram_tensor("pos", [N, D], FP, kind="ExternalInput")
    n1w_d = nc.dram_tensor("norm1_w", [D], FP, kind="ExternalInput")
    n2w_d = nc.dram_tensor("norm2_w", [D], FP, kind="ExternalInput")
    w1_d = nc.dram_tensor("w1_w", [DH, D], FP, kind="ExternalInput")
    b1_d = nc.dram_tensor("w1_b", [DH], FP, kind="ExternalInput")
    w2_d = nc.dram_tensor("w2_w", [DH, D], FP, kind="ExternalInput")
    b2_d = nc.dram_tensor("w2_b", [DH], FP, kind="ExternalInput")
    w3_d = nc.dram_tensor("w3_w", [D, DH], FP, kind="ExternalInput")
    b3_d = nc.dram_tensor("w3_b", [D], FP, kind="ExternalInput")
    out_d = nc.dram_tensor("out", [NPC, D], FP, kind="ExternalOutput")
    xr_dram = nc.dram_tensor("xr_own", [NPC, D], FP)

    tensors = (x_d, pos_d, n1w_d, n2w_d, w1_d, b1_d, w2_d, b2_d, w3_d, b3_d,
               out_d, xr_dram)
    with tile.TileContext(nc) as tc:
        with ExitStack() as ctx:
            build_kernel_body(nc, tc, ctx, tensors)
    nc.compile()
    return nc


_NC_CACHE = None


def _get_nc():
    global _NC_CACHE
    if _NC_CACHE is None:
        _NC_CACHE = build_program()
    return _NC_CACHE


def make_in_maps(inputs):
    in_maps = []
    for c in range(NCORES):
        sh = c * NPC
        m = {
            "x": np.ascontiguousarray(np.roll(inputs["x"], -sh, axis=0)),
            "pos": np.ascontiguousarray(np.roll(inputs["pos"], -sh, axis=0)),
            "norm1_w": inputs["norm1_w"], "norm2_w": inputs["norm2_w"],
            "w1_w": inputs["w1_w"], "w1_b": inputs["w1_b"],
            "w2_w": inputs["w2_w"], "w2_b": inputs["w2_b"],
            "w3_w": inputs["w3_w"], "w3_b": inputs["w3_b"],
        }
        in_maps.append({k: np.asarray(v, dtype=np.float32) for k, v in m.items()})
    return in_maps


def run_on_hw(inputs, trace=False):
    from concourse.bass_utils import run_bass_kernel_spmd
    nc = _get_nc()
    res = run_bass_kernel_spmd(nc, make_in_maps(inputs), list(range(NCORES)),
                               trace=trace)
    out = np.concatenate([res.results[c]["out"] for c in range(NCORES)], axis=0)
    return out, res


def kernel(**inputs):
    out, _ = run_on_hw(inputs, trace=False)
    return out



# revision 56
# speedup vs baseline: 1.0111x; 1.0111x over previous
"""Trainium2 Bass kernel for nn_BasicLayer (Erwin NSA-MSA sparse ball attention).

8 NeuronCores, data-parallel over points: each core receives the full x/pos
ROTATED so its own 1024 points sit at rows [0:1024] (whole-ball rotation makes
one SPMD program correct for every core; no collectives).

Per core:
  - Stage A: xr = rmsnorm(x)*n1w + rel for all 8192 points, ball-per-partition;
    bf16 copy of xr kept as the gather source; ball-summary keys by reduction.
    Ball pos-means via flat halving folds; rsqrt = exp(-0.5*ln(v)) + Newton
    (keeps ACT in the natural_log_exp table set used by attention exp).
  - Routing logits via 4-term bf16-split PE matmuls (bit-exact vs fp32 ref, so
    top-2 ball selection matches jax.lax.top_k except true fp32 ties).
  - Per (head, tile): DVE max8 -> equality masks [n,b] in bf16 -> PE transpose
    -> PE one-hot gather matmuls -> single PSUM->SBUF bf16 evac (ACT).
  - Per head, batched across tiles (DVE tensor_reduce runs at 1x, so all
    reductions are pairwise halving folds that hit the 2x bf16/fp16 TT mode):
    scores product (2x: e-innermost broadcast AP packs) + 3 e-pair-folds;
    exp emitted e-REPLICATED by ACT (step-0 input re-read) so the
    weighted-sum product is a flat 2x TT, in tile-halves for ACT/DVE overlap;
    z by strided reduce of one e-lane; (k,m)-reduce via contiguous [m,e]-run
    halving folds in fp16.
  - Residual + RMSNorm + SwiGLU MLP (fp16 weights/transposes/matmuls on PE --
    fp32 PE runs at quarter rate; Silu in one ACT op, emitted last so its
    table set loads once).
"""

import numpy as np

import concourse.bacc as bacc
import concourse.bass as bass
import concourse.mybir as mybir
import concourse.tile as tile
from concourse.masks import make_identity

FP = mybir.dt.float32
BF = mybir.dt.bfloat16
F16 = mybir.dt.float16
U16 = mybir.dt.uint16
I16 = mybir.dt.int16

N, D = 8192, 64
M = 64          # ball size
NB = N // M     # 128 balls
H, EH = 8, 8
TOPK = 2
NCORES = 8
NPC = N // NCORES   # 1024 points per core
NT = NPC // 128     # 8 point-tiles of 128
BPC = NPC // M      # 16 own balls per core
DH = D * 4          # 256 mlp hidden
EPS = 1.1920929e-07
ISQ8 = float(1.0 / np.sqrt(EH))
EM = M * EH         # 512 = gathered elem size (m-major, e innermost)
NG = NT * TOPK      # 16 gather slots per point
NIDX = NG * 128     # 2048 gathered blocks per head

A = mybir.AluOpType
AF = mybir.ActivationFunctionType
AX = mybir.AxisListType


def _bc(ap, dim, count):
    """Insert a step-0 (broadcast) dim at position `dim` of an AP."""
    new = [list(p) for p in ap.ap]
    new.insert(dim, [0, count])
    return bass.AP(tensor=ap.tensor, offset=ap.offset, ap=new)


def build_kernel_body(nc, tc, ctx, tensors):
    (x_d, pos_d, n1w_d, n2w_d, w1_d, b1_d, w2_d, b2_d, w3_d, b3_d,
     out_d, xr_dram) = tensors

    consts = ctx.enter_context(tc.tile_pool(name="consts", bufs=1))
    big = ctx.enter_context(tc.tile_pool(name="big", bufs=1))
    front_cm = tc.tile_pool(name="front", bufs=1)
    front = front_cm.__enter__()
    ps_tr_cm = tc.tile_pool(name="ps_tr", bufs=2, space="PSUM")
    ps_tr = ps_tr_cm.__enter__()

    ident = consts.tile([128, 128], FP)
    make_identity(nc, ident)


    # ---------------- Stage A: load + xr = rmsnorm(x)*n1w + rel (ball-major) ----
    x_bm = front.tile([128, M, D], FP)       # [ball, m, d]
    pos_bm = front.tile([128, M, D], FP)
    # x and pos on different DMA queues so the two 2MB loads overlap
    nc.sync.dma_start(out=x_bm, in_=x_d.ap().rearrange("(b m) d -> b m d", m=M))
    nc.gpsimd.dma_start(out=pos_bm,
                        in_=pos_d.ap().rearrange("(b m) d -> b m d", m=M))

    n1w_sb = consts.tile([128, D], FP)
    nc.sync.dma_start(out=n1w_sb,
                      in_=bass.AP(tensor=n1w_d, offset=0, ap=[[0, 128], [1, D]]))

    # ball mean of pos (over m): flat contiguous halving folds (m-major)
    mpf = front.tile([128, 32, D], FP, tag="mpf")
    nc.vector.tensor_tensor(out=mpf, in0=pos_bm[:, 0:32, :],
                            in1=pos_bm[:, 32:64, :], op=A.add)
    w = 16
    while w >= 1:
        nc.vector.tensor_tensor(out=mpf[:, 0:w, :], in0=mpf[:, 0:w, :],
                                in1=mpf[:, w:2 * w, :], op=A.add)
        w //= 2
    mp = front.tile([128, D], FP, tag="mp")
    nc.vector.tensor_scalar(mp, mpf[:, 0, :], 1.0 / M, None, op0=A.mult)

    # rms: 1/sqrt(mean(x^2) + eps)
    sq = front.tile([128, M, D], FP, tag="sq")
    nc.scalar.activation(out=sq, in_=x_bm, func=AF.Square)
    sq8 = front.tile([128, M, 8], FP, tag="sq8")
    nc.vector.tensor_reduce(out=sq8, in_=sq.rearrange("b m (g d) -> b m g d", g=8),
                            axis=AX.X, op=A.add)
    msq = front.tile([128, M], FP, tag="msq")
    nc.vector.tensor_reduce(out=msq, in_=sq8, axis=AX.X, op=A.add)
    nc.vector.tensor_scalar(msq, msq, 1.0 / D, EPS, op0=A.mult, op1=A.add)
    rinv = front.tile([128, M], FP, tag="rinv")
    lnv = front.tile([128, M], FP, tag="lnv")
    nc.scalar.activation(out=lnv, in_=msq, func=AF.Ln)
    nc.scalar.activation(out=rinv, in_=lnv, func=AF.Exp, scale=-0.5)
    # one Newton step: r <- r*(1.5 - 0.5*msq*r^2)
    rsqv = front.tile([128, M], FP, tag="rsqv")
    nc.vector.tensor_tensor(out=rsqv, in0=rinv, in1=rinv, op=A.mult)
    nc.vector.tensor_tensor(out=rsqv, in0=rsqv, in1=msq, op=A.mult)
    nc.vector.tensor_scalar(rsqv, rsqv, -0.5, 1.5, op0=A.mult, op1=A.add)
    nc.vector.tensor_tensor(out=rinv, in0=rinv, in1=rsqv, op=A.mult)

    nc.vector.tensor_tensor(out=pos_bm, in0=pos_bm, in1=_bc(mp, 1, M),
                            op=A.subtract)      # pos_bm becomes rel
    xr_bm = front.tile([128, M, D], FP)
    nc.vector.tensor_tensor(out=xr_bm, in0=x_bm, in1=_bc(rinv, 2, D), op=A.mult)
    nc.vector.tensor_tensor(out=xr_bm, in0=xr_bm, in1=_bc(n1w_sb, 1, M), op=A.mult)
    nc.vector.tensor_tensor(out=xr_bm, in0=xr_bm, in1=pos_bm, op=A.add)

    # ---------------- Stage B: KB to DRAM (bf16) + ball-summary keys ----------
    xr16 = big.tile([128, M, D], BF)
    nc.vector.tensor_copy(out=xr16, in_=xr_bm)

    # ball-summary keys: flat halving folds over m (strided reduce is 2x slower)
    kf = front.tile([128, 32, D], FP, tag="kf")
    nc.vector.tensor_tensor(out=kf, in0=xr_bm[:, 0:32, :],
                            in1=xr_bm[:, 32:64, :], op=A.add)
    w = 16
    while w >= 1:
        nc.vector.tensor_tensor(out=kf[:, 0:w, :], in0=kf[:, 0:w, :],
                                in1=kf[:, w:2 * w, :], op=A.add)
        w //= 2
    keys_bm = kf[:, 0, :]
    keysT = front.tile([64, 128], FP)                 # [(h e), ball]
    kt_ps = ps_tr.tile([64, 128], FP, tag="tr")
    nc.tensor.transpose(kt_ps, keys_bm, ident)
    nc.scalar.copy(out=keysT, in_=kt_ps)

    # ---------------- Stage C: own-point layouts --------------------------------
    # xr rows [0:1024] -> DRAM bounce -> point-major + transposed copies
    nc.sync.dma_start(out=xr_dram.ap().rearrange("(b m) d -> b m d", m=M),
                      in_=xr_bm[0:BPC, :, :])
    q_pm = big.tile([128, NT, H, EH], FP)  # per-partition q scalars
    nc.sync.dma_start(out=q_pm,
                      in_=xr_dram.ap().rearrange("(t p) (h e) -> p t h e", p=128, e=EH))
    x_own = big.tile([128, NT, D], FP)
    nc.sync.dma_start(out=x_own,
                      in_=x_d.ap()[0:NPC, :].rearrange("(t p) d -> p t d", p=128))

    qT = front.tile([64, NT, 128], FP)       # [(h e), nt, n128]
    for t in range(NT):
        q_ps = ps_tr.tile([64, 128], FP, tag="tr")
        nc.tensor.transpose(q_ps, q_pm[:, t].rearrange("p h e -> p (h e)"), ident)
        nc.scalar.copy(out=qT[:, t, :], in_=q_ps)

    # hi/lo bf16 split: 4-term split q.k = qhi.khi + qhi.klo + qlo.khi + qlo.klo
    # -> every bf16 product exact in fp32 -> logits match fp32 ref to ~2 ulp.
    # K-row order p = e*4 + j ; k terms [khi, klo, khi, klo], q [qhi, qhi, qlo, qlo]
    identB = consts.tile([128, 128], BF)
    nc.vector.tensor_copy(out=identB, in_=ident)
    kst4 = front.tile([64, 4, 128], BF)
    qst4 = front.tile([64, 4, NT, 128], BF)
    tmp32 = front.tile([64, NT, 128], FP, tag="tmp32")
    for (src_ap, dst, nfree) in ((keysT, kst4, 1), (qT, qst4, NT)):
        nc.vector.tensor_copy(out=dst[:, 0], in_=src_ap)     # hi (cast bf16)
        t32 = tmp32[:, 0:nfree, :] if nfree == NT else tmp32[:, 0, :]
        nc.vector.tensor_copy(out=t32, in_=dst[:, 0])        # hi back to fp32
        nc.vector.tensor_tensor(out=t32, in0=src_ap, in1=t32, op=A.subtract)
        nc.vector.tensor_copy(out=dst[:, 1], in_=t32)        # lo (cast bf16)
        if nfree == 1:   # k: [khi, klo, khi, klo]
            nc.vector.tensor_copy(out=dst[:, 2], in_=dst[:, 0])
            nc.vector.tensor_copy(out=dst[:, 3], in_=dst[:, 1])
        else:            # q: [qhi, qhi, qlo, qlo]
            nc.vector.tensor_copy(out=dst[:, 2], in_=dst[:, 1])
            nc.vector.tensor_copy(out=dst[:, 3], in_=dst[:, 1])
            nc.vector.tensor_copy(out=dst[:, 1], in_=dst[:, 0])
    kstack = big.tile([32, H, 128], BF)
    qstack = big.tile([32, H, NT, 128], BF)
    for h in range(H):
        # two DMA queues so the 16 strided loads overlap (head order kept)
        nc.sync.dma_start(
            out=kstack[:, h, :],
            in_=bass.AP(tensor=kst4.tensor, offset=kst4.offset + 8 * h * 4 * 128,
                        ap=[[4 * 128, 8], [128, 4], [1, 128]]))
        nc.gpsimd.dma_start(
            out=qstack[:, h],
            in_=bass.AP(tensor=qst4.tensor,
                        offset=qst4.offset + 8 * h * 4 * NT * 128,
                        ap=[[4 * NT * 128, 8], [NT * 128, 4], [128, NT],
                            [1, 128]]))
    # weights: w1/w2 [256, 64] -> transposed fp16 [64, 256]; w3 [64, 256] ->
    # [256, 64] (fp16: fp32 PE matmuls/transposes run at quarter rate)
    w1n = consts.tile([128, 2, D], FP)
    w2n = consts.tile([128, 2, D], FP)
    w3n = consts.tile([64, DH], FP)
    nc.sync.dma_start(out=w1n, in_=w1_d.ap().rearrange("(c j) d -> j c d", j=128))
    nc.sync.dma_start(out=w2n, in_=w2_d.ap().rearrange("(c j) d -> j c d", j=128))
    nc.sync.dma_start(out=w3n, in_=w3_d.ap())
    w1h = consts.tile([128, 2, D], F16)
    w2h = consts.tile([128, 2, D], F16)
    w3h = consts.tile([64, DH], F16)
    nc.vector.tensor_copy(out=w1h, in_=w1n)
    nc.vector.tensor_copy(out=w2h, in_=w2n)
    nc.vector.tensor_copy(out=w3h, in_=w3n)
    identH = consts.tile([128, 128], F16)
    nc.vector.tensor_copy(out=identH, in_=ident)
    w1T = consts.tile([64, DH], F16)   # [d, j]
    w2T = consts.tile([64, DH], F16)
    w3T = consts.tile([128, 2, D], F16)  # [j128, c, e]
    for c in range(2):
        t_ps = ps_tr.tile([64, 128], F16, tag="trh")
        nc.tensor.transpose(t_ps, w1h[:, c, :], identH)
        nc.scalar.copy(out=w1T[:, 128 * c:128 * (c + 1)], in_=t_ps)
        t_ps2 = ps_tr.tile([64, 128], F16, tag="trh")
        nc.tensor.transpose(t_ps2, w2h[:, c, :], identH)
        nc.scalar.copy(out=w2T[:, 128 * c:128 * (c + 1)], in_=t_ps2)
        t_ps3 = ps_tr.tile([128, 64], F16, tag="trh2")
        nc.tensor.transpose(t_ps3, w3h[:, 128 * c:128 * (c + 1)],
                            identH[0:64, 0:64])
        nc.scalar.copy(out=w3T[:, c, :], in_=t_ps3)
    b1_sb = consts.tile([128, 2], FP)
    b2_sb = consts.tile([128, 2], FP)
    nc.sync.dma_start(out=b1_sb, in_=b1_d.ap().rearrange("(c j) -> j c", j=128))
    nc.sync.dma_start(out=b2_sb, in_=b2_d.ap().rearrange("(c j) -> j c", j=128))
    b3_sb = consts.tile([128, D], FP)
    nc.sync.dma_start(out=b3_sb,
                      in_=bass.AP(tensor=b3_d, offset=0, ap=[[0, 128], [1, D]]))

    front_cm.__exit__(None, None, None)
    ps_tr_cm.__exit__(None, None, None)

    # ---------------- Stage D+E: selection + PE one-hot gather + attention -----
    # Per (h, t): PE logits -> DVE max8 -> DVE equality masks [n, b] (bf16)
    # -> PE transpose -> evac maskT -> PE gather matmuls -> evac G to bf16.
    # Then per-head batched DVE scores/softmax/weighted-sum (as v2).
    gpool = ctx.enter_context(tc.tile_pool(name="gpool", bufs=4))
    wpool = ctx.enter_context(tc.tile_pool(name="wpool", bufs=2))
    sel_cm = tc.tile_pool(name="sel", bufs=8)
    sel = sel_cm.__enter__()
    ps_lt_cm = tc.tile_pool(name="ps_lt", bufs=2, space="PSUM")
    ps_lt = ps_lt_cm.__enter__()
    ps_mt_cm = tc.tile_pool(name="ps_mt", bufs=2, space="PSUM")
    ps_mt = ps_mt_cm.__enter__()
    ps_g_cm = tc.tile_pool(name="ps_g", bufs=2, space="PSUM")
    ps_g = ps_g_cm.__enter__()

    attn16 = big.tile([128, NT, H, EH], BF)
    zh = big.tile([128, H, NT], FP)

    def selection(h, ts):
        """Logits (PE) + top-2 masks (DVE) for head h, tiles ts; returns masks.

        Split from the gather so its DVE ops can be emitted while head h-1's
        ACT exp runs, and the gather's ACT ops (maskT + evac) can be emitted
        AFTER head h-1's exp — engines are strict FIFO, so emission order on
        each engine is execution order."""
        masks = {}
        for t in ts:
            lpm_ps = ps_lt.tile([128, 128], FP, tag="lt")
            nc.tensor.matmul(lpm_ps, lhsT=qstack[:, h, t, :],
                             rhs=kstack[:, h, :], start=True, stop=True)
            v8t = sel.tile([128, 8], FP, tag="v8")
            nc.vector.max(out=v8t, in_=lpm_ps)
            # both tk masks in one op: mask2[p, tk, b] = (lpm == v8[tk])
            mask2 = sel.tile([128, TOPK, 128], BF, tag="mnb")
            nc.vector.tensor_tensor(
                out=mask2, in0=_bc(lpm_ps, 1, TOPK),
                in1=bass.AP(tensor=v8t.tensor, offset=v8t.offset,
                            ap=[v8t.ap[0], [1, TOPK], [0, 128]]),
                op=A.is_equal)
            masks[t] = mask2
        return masks

    def gather_head(h, masks, ts, g_sb=None):
        """PE transposes + one-hot gathers + ACT evacs for head h, tiles ts.

        maskT(t+1) is emitted BEFORE evac(t) so the strict-FIFO ACT queue
        doesn't serialize ACT->PE->ACT per tile: gather(t) runs on PE while
        ACT does maskT(t+1), and evac(t) follows immediately."""
        if g_sb is None:
            g_sb = gpool.tile([128, NT, TOPK, M, EH], BF, tag="g")

        def emit_maskT_gather(t):
            mask2 = masks[t]
            mt_ps = ps_mt.tile([128, TOPK, 128], BF, tag="mt")
            for tk in range(TOPK):
                nc.tensor.transpose(mt_ps[:, tk, :], mask2[:, tk, :], identB)
            maskT = sel.tile([128, TOPK, 128], BF, tag="mT")
            nc.scalar.copy(out=maskT, in_=mt_ps)
            g_ps = ps_g.tile([128, TOPK, M, EH], FP, tag="g")
            for tk in range(TOPK):
                # rhs: xr16 [b, m, (h e)] -> per-head (m, e) column order
                nc.tensor.matmul(
                    g_ps[:, tk].rearrange("p m e -> p (m e)"),
                    lhsT=maskT[:, tk, :],
                    rhs=bass.AP(tensor=xr16.tensor,
                                offset=xr16.offset + EH * h,
                                ap=[xr16.ap[0], [D, M], [1, EH]]),
                    start=True, stop=True)
            return g_ps

        gp_prev, t_prev = emit_maskT_gather(ts[0]), ts[0]
        for t in ts[1:]:
            gp = emit_maskT_gather(t)
            # evac PSUM -> bf16 SBUF on ACT (DVE is the bottleneck engine)
            nc.scalar.copy(out=g_sb[:, t_prev], in_=gp_prev)
            gp_prev, t_prev = gp, t
        nc.scalar.copy(out=g_sb[:, t_prev], in_=gp_prev)
        return g_sb

    def compute_a(h, g_sb):
        """Scores + batched exp for head h (DVE prod/e-folds, ACT exp)."""
        nt = NT
        ng = nt * TOPK
        # bf16 q for this head, replicated over topk: [p, (t k), e]
        q2h = wpool.tile([128, nt, TOPK, EH], BF, tag="q2")
        nc.vector.tensor_copy(
            out=q2h,
            in_=bass.AP(tensor=q_pm.tensor,
                        offset=q_pm.offset + EH * h,
                        ap=[q_pm.ap[0], [H * EH, nt], [0, TOPK], [1, EH]]))
        g_v = g_sb.rearrange("p t k m e -> p (t k) m e")
        prod = wpool.tile([128, ng, M, EH], BF, tag="prod")
        q2_bc = bass.AP(tensor=q2h.tensor, offset=q2h.offset,
                        ap=[q2h.ap[0], [EH, ng], [0, M], [1, EH]])
        nc.vector.tensor_tensor(out=prod, in0=g_v, in1=q2_bc, op=A.mult)
        # s[p, (g m)] fp16 = sum_e prod, via 3 pair-folds over e (e pairs are
        # step-1 runs of 4/2/1 -> cheaper than the 1x-mode strided reduce).
        # scr16 is shared by s4 (e-folds) and p2k (m-folds) — disjoint lifetimes
        scr16 = wpool.tile([128, NT * M * EH], F16, tag="scr")
        s4 = scr16[:, 0:ng * M * 4].rearrange("p (j f) -> p j f", f=4)
        nc.vector.tensor_tensor(
            out=s4,
            in0=bass.AP(tensor=prod.tensor, offset=prod.offset,
                        ap=[prod.ap[0], [EH, ng * M], [1, 4]]),
            in1=bass.AP(tensor=prod.tensor, offset=prod.offset + 4,
                        ap=[prod.ap[0], [EH, ng * M], [1, 4]]),
            op=A.add)
        nc.vector.tensor_tensor(
            out=bass.AP(tensor=s4.tensor, offset=s4.offset,
                        ap=[s4.ap[0], [4, ng * M], [1, 2]]),
            in0=bass.AP(tensor=s4.tensor, offset=s4.offset,
                        ap=[s4.ap[0], [4, ng * M], [1, 2]]),
            in1=bass.AP(tensor=s4.tensor, offset=s4.offset + 2,
                        ap=[s4.ap[0], [4, ng * M], [1, 2]]),
            op=A.add)
        s_sb = wpool.tile([128, ng, M], F16, tag="s")
        nc.vector.tensor_tensor(
            out=s_sb,
            in0=bass.AP(tensor=s4.tensor, offset=s4.offset,
                        ap=[s4.ap[0], [4, ng * M]]),
            in1=bass.AP(tensor=s4.tensor, offset=s4.offset + 1,
                        ap=[s4.ap[0], [4, ng * M]]),
            op=A.add)
        # softmax numerator: p = exp(s/sqrt8), written REPLICATED over e by
        # re-reading s 8x (step-0 input dim) -> prod2 becomes a flat 2x TT.
        # Emitted in tile-halves so ACT exp overlaps DVE prod2 in compute_b.
        p_rep = wpool.tile([128, ng, M, EH], BF, tag="prep")
        hj = ng * M // 2
        for hf in range(2):
            sl = slice(hf * hj, (hf + 1) * hj)
            nc.scalar.activation(
                out=p_rep.rearrange("p g m e -> p (g m) e")[:, sl],
                in_=bass.AP(tensor=s_sb.tensor, offset=s_sb.offset + hf * hj,
                            ap=[s_sb.ap[0], [1, hj], [0, EH]]),
                func=AF.Exp, scale=ISQ8)
        return g_v, p_rep, scr16

    def compute_b(h, state, t0=0, t1=NT):
        """Weighted sum + z + (k,m)-folds for head h, tiles [t0:t1) (DVE)."""
        g_v, p_rep, scr16 = state
        nt = t1 - t0
        ng = nt * TOPK
        j0 = t0 * TOPK * M                 # (g m)-flat element base
        base = j0 * EH                     # fully-flat element base
        prod2 = wpool.tile([128, NT * TOPK, M, EH], BF, tag="prod")
        hj = ng * M // 2
        for hf in range(2):
            sl = slice(j0 + hf * hj, j0 + (hf + 1) * hj)
            nc.vector.tensor_tensor(
                out=prod2.rearrange("p g m e -> p (g m) e")[:, sl],
                in0=g_v.rearrange("p g m e -> p (g m) e")[:, sl],
                in1=p_rep.rearrange("p g m e -> p (g m) e")[:, sl],
                op=A.mult)
        # z[p, t] = sum over (tk, m) of p (read one e-lane of the replica)
        nc.vector.tensor_reduce(
            out=zh[:, h, t0:t1],
            in_=bass.AP(tensor=p_rep.tensor, offset=p_rep.offset + base,
                        ap=[p_rep.ap[0], [TOPK * M * EH, nt], [EH, TOPK * M]]),
            axis=AX.X, op=A.add)
        # fold the two topk slots with a packed TT-add, then reduce over m by
        # halving folds on contiguous [m, e] runs (2x-eligible, vs 1x reduce)
        p2k = scr16[:, t0 * M * EH:t1 * M * EH].rearrange(
            "p (t m e) -> p t m e", m=M, e=EH)
        nc.vector.tensor_tensor(
            out=p2k,
            in0=bass.AP(tensor=prod2.tensor, offset=prod2.offset + base,
                        ap=[prod2.ap[0], [TOPK * M * EH, nt], [EH, M], [1, EH]]),
            in1=bass.AP(tensor=prod2.tensor,
                        offset=prod2.offset + base + M * EH,
                        ap=[prod2.ap[0], [TOPK * M * EH, nt], [EH, M], [1, EH]]),
            op=A.add)
        w = (M // 2) * EH
        while w > EH:
            nc.vector.tensor_tensor(
                out=bass.AP(tensor=p2k.tensor, offset=p2k.offset,
                            ap=[p2k.ap[0], [M * EH, nt], [1, w]]),
                in0=bass.AP(tensor=p2k.tensor, offset=p2k.offset,
                            ap=[p2k.ap[0], [M * EH, nt], [1, w]]),
                in1=bass.AP(tensor=p2k.tensor, offset=p2k.offset + w,
                            ap=[p2k.ap[0], [M * EH, nt], [1, w]]),
                op=A.add)
            w //= 2
        nc.vector.tensor_tensor(
            out=attn16[:, t0:t1, h, :],
            in0=bass.AP(tensor=p2k.tensor, offset=p2k.offset,
                        ap=[p2k.ap[0], [M * EH, nt], [1, EH]]),
            in1=bass.AP(tensor=p2k.tensor, offset=p2k.offset + EH,
                        ap=[p2k.ap[0], [M * EH, nt], [1, EH]]),
            op=A.add)

    # finalize tiles live in `big` so the per-half epilogue can interleave
    # with head 7's split compute (attention pools still open)
    zinv = big.tile([128, H, NT], FP)
    attn = big.tile([128, NT, D], FP)
    y = big.tile([128, NT, D], FP)
    sq2 = big.tile([128, NT, D], FP)
    ri2 = big.tile([128, NT], FP)
    ln2 = big.tile([128, NT], FP)
    x2 = big.tile([128, NT, D], F16)
    n2w_sb = consts.tile([128, D], FP)
    nc.sync.dma_start(out=n2w_sb,
                      in_=bass.AP(tensor=n2w_d, offset=0, ap=[[0, 128], [1, D]]))

    def finalize_a(t0, t1):
        """attn normalize + residual + rmsnorm2 + x2 for tiles [t0:t1)."""
        nt = t1 - t0
        nc.vector.reciprocal(out=zinv[:, :, t0:t1], in_=zh[:, :, t0:t1])
        zinv_bc = bass.AP(tensor=zinv.tensor, offset=zinv.offset + t0,
                          ap=[zinv.ap[0], [1, nt], [NT, H], [0, EH]])
        nc.vector.tensor_tensor(
            out=attn.rearrange("p t (h e) -> p t h e", e=EH)[:, t0:t1],
            in0=attn16[:, t0:t1], in1=zinv_bc, op=A.mult)
        nc.vector.tensor_tensor(out=y[:, t0:t1], in0=x_own[:, t0:t1],
                                in1=attn[:, t0:t1], op=A.add)
        nc.scalar.activation(out=sq2[:, t0:t1], in_=y[:, t0:t1], func=AF.Square)
        nc.vector.tensor_reduce(out=ri2[:, t0:t1], in_=sq2[:, t0:t1],
                                axis=AX.X, op=A.add)
        nc.vector.tensor_scalar(ri2[:, t0:t1], ri2[:, t0:t1], 1.0 / D, EPS,
                                op0=A.mult, op1=A.add)
        nc.scalar.activation(out=ln2[:, t0:t1], in_=ri2[:, t0:t1], func=AF.Ln)
        nc.scalar.activation(out=ri2[:, t0:t1], in_=ln2[:, t0:t1],
                             func=AF.Exp, scale=-0.5)
        nc.vector.tensor_tensor(out=x2[:, t0:t1], in0=y[:, t0:t1],
                                in1=_bc(ri2[:, t0:t1], 2, D), op=A.mult)
        nc.vector.tensor_tensor(out=x2[:, t0:t1], in0=x2[:, t0:t1],
                                in1=_bc(n2w_sb, 1, nt), op=A.mult)

    # Software pipeline over heads. Emission order IS execution order on each
    # strict-FIFO engine, so head h+1's selection/gather is emitted in two
    # tile-halves AROUND head h's exp: ACT runs [mT/evac(h+1) 0-3, exp(h),
    # mT/evac(h+1) 4-7] while DVE runs [masks(h+1) 0-3, prod(h), e-folds(h),
    # masks(h+1) 4-7, prod2(h), folds(h)] with no exp-wait bubble.
    half0, half1 = list(range(NT // 2)), list(range(NT // 2, NT))
    g_cur = gather_head(0, selection(0, half0 + half1), half0 + half1)
    for h in range(H - 1):
        m0 = selection(h + 1, half0)
        g_next = gather_head(h + 1, m0, half0)
        st = compute_a(h, g_cur)
        m1 = selection(h + 1, half1)
        gather_head(h + 1, m1, half1, g_sb=g_next)
        compute_b(h, st)
        g_cur = g_next
    st_last = compute_a(H - 1, g_cur)
    ps_g_cm.__exit__(None, None, None)
    ps_mt_cm.__exit__(None, None, None)
    ps_lt_cm.__exit__(None, None, None)
    sel_cm.__exit__(None, None, None)

    mlpw = ctx.enter_context(tc.tile_pool(name="mlpw", bufs=1))
    ps_m = ctx.enter_context(tc.tile_pool(name="ps_m", bufs=1, space="PSUM"))
    x2T = mlpw.tile([64, NT, 128], F16)
    hhT = mlpw.tile([128, 2, NT, 128], F16)   # [j128, c, nt, n]
    final = mlpw.tile([128, NT, D], FP)

    def mlp_half(t0, t1):
        """x2T transposes + SwiGLU MLP (fp16) + residual + out DMA for [t0:t1)."""
        nt = t1 - t0
        for t in range(t0, t1):
            xt_ps = ps_m.tile([64, 128], F16, tag="tr")
            nc.tensor.transpose(xt_ps, x2[:, t, :], identH)
            nc.scalar.copy(out=x2T[:, t, :], in_=xt_ps)
        nc.vector.tensor_tensor(out=y[:, t0:t1], in0=y[:, t0:t1],
                                in1=_bc(b3_sb, 1, nt), op=A.add)
        for c in range(2):
            h1_ps = ps_m.tile([128, nt, 128], FP, tag="h1")
            h2_ps = ps_m.tile([128, nt, 128], FP, tag="h2")
            nc.tensor.matmul(h1_ps.rearrange("j t n -> j (t n)"),
                             lhsT=w1T[:, 128 * c:128 * (c + 1)],
                             rhs=x2T[:, t0:t1].rearrange("d t n -> d (t n)"),
                             start=True, stop=True)
            nc.tensor.matmul(h2_ps.rearrange("j t n -> j (t n)"),
                             lhsT=w2T[:, 128 * c:128 * (c + 1)],
                             rhs=x2T[:, t0:t1].rearrange("d t n -> d (t n)"),
                             start=True, stop=True)
            h1b = mlpw.tile([128, nt, 128], F16, tag="h1b")
            nc.vector.tensor_scalar(h1b, h1_ps, b1_sb[:, c:c + 1], None,
                                    op0=A.add)
            sgm = mlpw.tile([128, nt, 128], F16, tag="sgm")
            nc.scalar.activation(out=sgm, in_=h1b, func=AF.Silu)
            h2s = mlpw.tile([128, nt, 128], F16, tag="h2s")
            nc.vector.tensor_scalar(h2s, h2_ps, b2_sb[:, c:c + 1], None,
                                    op0=A.add)
            nc.vector.tensor_tensor(out=hhT[:, c, t0:t1], in0=sgm, in1=h2s,
                                    op=A.mult)
        for t in range(t0, t1):
            o_ps = ps_m.tile([128, D], FP, tag="o")
            for c in range(2):
                nc.tensor.matmul(o_ps, lhsT=hhT[:, c, t, :], rhs=w3T[:, c, :],
                                 start=(c == 0), stop=(c == 1))
            nc.vector.scalar_tensor_tensor(out=final[:, t], in0=o_ps,
                                           scalar=1.0, in1=y[:, t],
                                           op0=A.mult, op1=A.add)
        nc.sync.dma_start(
            out=out_d.ap().rearrange("(t p) d -> p t d", p=128)[:, t0:t1],
            in_=final[:, t0:t1])

    # Split epilogue: head 7's second-half folds overlap finalize of the
    # first half; the MLP halves pipeline behind finalize.
    compute_b(H - 1, st_last, 0, NT // 2)
    finalize_a(0, NT // 2)
    compute_b(H - 1, st_last, NT // 2, NT)
    finalize_a(NT // 2, NT)
    mlp_half(0, NT // 2)
    mlp_half(NT // 2, NT)


def build_program():
    from contextlib import ExitStack
    nc = bacc.Bacc("TRN2", target_bir_lowering=False, debug=False,
                   num_devices=NCORES)
    x_d = nc.dram_tensor("x", [N, D], FP, kind="ExternalInput")
    pos_d = nc.dram_tensor("pos", [N, D], FP, kind="ExternalInput")
    n1w_d = nc.dram_tensor("norm1_w", [D], FP, kind="ExternalInput")
    n2w_d = nc.dram_tensor("norm2_w", [D], FP, kind="ExternalInput")
    w1_d = nc.dram_tensor("w1_w", [DH, D], FP, kind="ExternalInput")
    b1_d = nc.dram_tensor("w1_b", [DH], FP, kind="ExternalInput")
    w2_d = nc.dram_tensor("w2_w", [DH, D], FP, kind="ExternalInput")
    b2_d = nc.dram_tensor("w2_b", [DH], FP, kind="ExternalInput")
    w3_d = nc.dram_tensor("w3_w", [D, DH], FP, kind="ExternalInput")
    b3_d = nc.dram_tensor("w3_b", [D], FP, kind="ExternalInput")
    out_d = nc.dram_tensor("out", [NPC, D], FP, kind="ExternalOutput")
    xr_dram = nc.dram_tensor("xr_own", [NPC, D], FP)

    tensors = (x_d, pos_d, n1w_d, n2w_d, w1_d, b1_d, w2_d, b2_d, w3_d, b3_d,
               out_d, xr_dram)
    with tile.TileContext(nc) as tc:
        with ExitStack() as ctx:
            build_kernel_body(nc, tc, ctx, tensors)
    nc.compile()
    return nc


_NC_CACHE = None


def _get_nc():
    global _NC_CACHE
    if _NC_CACHE is None:
        _NC_CACHE = build_program()
    return _NC_CACHE


def make_in_maps(inputs):
    in_maps = []
    for c in range(NCORES):
        sh = c * NPC
        m = {
            "x": np.ascontiguousarray(np.roll(inputs["x"], -sh, axis=0)),
            "pos": np.ascontiguousarray(np.roll(inputs["pos"], -sh, axis=0)),
            "norm1_w": inputs["norm1_w"], "norm2_w": inputs["norm2_w"],
            "w1_w": inputs["w1_w"], "w1_b": inputs["w1_b"],
            "w2_w": inputs["w2_w"], "w2_b": inputs["w2_b"],
            "w3_w": inputs["w3_w"], "w3_b": inputs["w3_b"],
        }
        in_maps.append({k: np.asarray(v, dtype=np.float32) for k, v in m.items()})
    return in_maps


def run_on_hw(inputs, trace=False):
    from concourse.bass_utils import run_bass_kernel_spmd
    nc = _get_nc()
    res = run_bass_kernel_spmd(nc, make_in_maps(inputs), list(range(NCORES)),
                               trace=trace)
    out = np.concatenate([res.results[c]["out"] for c in range(NCORES)], axis=0)
    return out, res


def kernel(**inputs):
    out, _ = run_on_hw(inputs, trace=False)
    return out



# revision 57
# speedup vs baseline: 1.0223x; 1.0111x over previous
"""Trainium2 Bass kernel for nn_BasicLayer (Erwin NSA-MSA sparse ball attention).

8 NeuronCores, data-parallel over points: each core receives the full x/pos
ROTATED so its own 1024 points sit at rows [0:1024] (whole-ball rotation makes
one SPMD program correct for every core; no collectives).

Per core:
  - Stage A: xr = rmsnorm(x)*n1w + rel for all 8192 points, ball-per-partition;
    bf16 copy of xr kept as the gather source; ball-summary keys by reduction.
    Ball pos-means via flat halving folds; rsqrt = exp(-0.5*ln(v)) + Newton
    (keeps ACT in the natural_log_exp table set used by attention exp).
  - Routing logits via 4-term bf16-split PE matmuls (bit-exact vs fp32 ref, so
    top-2 ball selection matches jax.lax.top_k except true fp32 ties).
  - Per (head, tile): DVE max8 -> equality masks [n,b] in bf16 -> PE transpose
    -> PE one-hot gather matmuls -> single PSUM->SBUF bf16 evac (ACT).
  - Per head, batched across tiles (DVE tensor_reduce runs at 1x, so all
    reductions are pairwise halving folds that hit the 2x bf16/fp16 TT mode):
    scores product (2x: e-innermost broadcast AP packs) + 3 e-pair-folds;
    exp emitted e-REPLICATED by ACT (step-0 input re-read) so the
    weighted-sum product is a flat 2x TT, in tile-halves for ACT/DVE overlap;
    z by strided reduce of one e-lane; (k,m)-reduce via contiguous [m,e]-run
    halving folds in fp16.
  - Residual + RMSNorm + SwiGLU MLP (fp16 weights/transposes/matmuls on PE --
    fp32 PE runs at quarter rate; Silu in one ACT op, emitted last so its
    table set loads once).
"""

import numpy as np

import concourse.bacc as bacc
import concourse.bass as bass
import concourse.mybir as mybir
import concourse.tile as tile
from concourse.masks import make_identity

FP = mybir.dt.float32
BF = mybir.dt.bfloat16
F16 = mybir.dt.float16
U16 = mybir.dt.uint16
I16 = mybir.dt.int16

N, D = 8192, 64
M = 64          # ball size
NB = N // M     # 128 balls
H, EH = 8, 8
TOPK = 2
NCORES = 8
NPC = N // NCORES   # 1024 points per core
NT = NPC // 128     # 8 point-tiles of 128
BPC = NPC // M      # 16 own balls per core
DH = D * 4          # 256 mlp hidden
EPS = 1.1920929e-07
ISQ8 = float(1.0 / np.sqrt(EH))
EM = M * EH         # 512 = gathered elem size (m-major, e innermost)
NG = NT * TOPK      # 16 gather slots per point
NIDX = NG * 128     # 2048 gathered blocks per head

A = mybir.AluOpType
AF = mybir.ActivationFunctionType
AX = mybir.AxisListType


def _bc(ap, dim, count):
    """Insert a step-0 (broadcast) dim at position `dim` of an AP."""
    new = [list(p) for p in ap.ap]
    new.insert(dim, [0, count])
    return bass.AP(tensor=ap.tensor, offset=ap.offset, ap=new)


def build_kernel_body(nc, tc, ctx, tensors):
    (x_d, pos_d, n1w_d, n2w_d, w1_d, b1_d, w2_d, b2_d, w3_d, b3_d,
     out_d, xr_dram) = tensors

    consts = ctx.enter_context(tc.tile_pool(name="consts", bufs=1))
    big = ctx.enter_context(tc.tile_pool(name="big", bufs=1))
    front_cm = tc.tile_pool(name="front", bufs=1)
    front = front_cm.__enter__()
    ps_tr_cm = tc.tile_pool(name="ps_tr", bufs=2, space="PSUM")
    ps_tr = ps_tr_cm.__enter__()

    ident = consts.tile([128, 128], FP)
    make_identity(nc, ident)


    # ---------------- Stage A: load + xr = rmsnorm(x)*n1w + rel (ball-major) ----
    x_bm = front.tile([128, M, D], FP)       # [ball, m, d]
    pos_bm = front.tile([128, M, D], FP)
    # x and pos on different DMA queues so the two 2MB loads overlap
    nc.sync.dma_start(out=x_bm, in_=x_d.ap().rearrange("(b m) d -> b m d", m=M))
    nc.gpsimd.dma_start(out=pos_bm,
                        in_=pos_d.ap().rearrange("(b m) d -> b m d", m=M))

    n1w_sb = consts.tile([128, D], FP)
    nc.sync.dma_start(out=n1w_sb,
                      in_=bass.AP(tensor=n1w_d, offset=0, ap=[[0, 128], [1, D]]))

    # ball mean of pos (over m): flat contiguous halving folds (m-major)
    mpf = front.tile([128, 32, D], FP, tag="mpf")
    nc.vector.tensor_tensor(out=mpf, in0=pos_bm[:, 0:32, :],
                            in1=pos_bm[:, 32:64, :], op=A.add)
    w = 16
    while w >= 1:
        nc.vector.tensor_tensor(out=mpf[:, 0:w, :], in0=mpf[:, 0:w, :],
                                in1=mpf[:, w:2 * w, :], op=A.add)
        w //= 2
    mp = front.tile([128, D], FP, tag="mp")
    nc.vector.tensor_scalar(mp, mpf[:, 0, :], 1.0 / M, None, op0=A.mult)

    # rms: 1/sqrt(mean(x^2) + eps)
    sq = front.tile([128, M, D], FP, tag="sq")
    nc.scalar.activation(out=sq, in_=x_bm, func=AF.Square)
    sq8 = front.tile([128, M, 8], FP, tag="sq8")
    nc.vector.tensor_reduce(out=sq8, in_=sq.rearrange("b m (g d) -> b m g d", g=8),
                            axis=AX.X, op=A.add)
    msq = front.tile([128, M], FP, tag="msq")
    nc.vector.tensor_reduce(out=msq, in_=sq8, axis=AX.X, op=A.add)
    nc.vector.tensor_scalar(msq, msq, 1.0 / D, EPS, op0=A.mult, op1=A.add)
    rinv = front.tile([128, M], FP, tag="rinv")
    lnv = front.tile([128, M], FP, tag="lnv")
    nc.scalar.activation(out=lnv, in_=msq, func=AF.Ln)
    nc.scalar.activation(out=rinv, in_=lnv, func=AF.Exp, scale=-0.5)
    # one Newton step: r <- r*(1.5 - 0.5*msq*r^2)
    rsqv = front.tile([128, M], FP, tag="rsqv")
    nc.vector.tensor_tensor(out=rsqv, in0=rinv, in1=rinv, op=A.mult)
    nc.vector.tensor_tensor(out=rsqv, in0=rsqv, in1=msq, op=A.mult)
    nc.vector.tensor_scalar(rsqv, rsqv, -0.5, 1.5, op0=A.mult, op1=A.add)
    nc.vector.tensor_tensor(out=rinv, in0=rinv, in1=rsqv, op=A.mult)

    nc.vector.tensor_tensor(out=pos_bm, in0=pos_bm, in1=_bc(mp, 1, M),
                            op=A.subtract)      # pos_bm becomes rel
    xr_bm = front.tile([128, M, D], FP)
    nc.vector.tensor_tensor(out=xr_bm, in0=x_bm, in1=_bc(rinv, 2, D), op=A.mult)
    nc.vector.tensor_tensor(out=xr_bm, in0=xr_bm, in1=_bc(n1w_sb, 1, M), op=A.mult)
    nc.vector.tensor_tensor(out=xr_bm, in0=xr_bm, in1=pos_bm, op=A.add)

    # ---------------- Stage B: KB to DRAM (bf16) + ball-summary keys ----------
    xr16 = big.tile([128, M, D], BF)
    nc.vector.tensor_copy(out=xr16, in_=xr_bm)

    # ball-summary keys: flat halving folds over m (strided reduce is 2x slower)
    kf = front.tile([128, 32, D], FP, tag="kf")
    nc.vector.tensor_tensor(out=kf, in0=xr_bm[:, 0:32, :],
                            in1=xr_bm[:, 32:64, :], op=A.add)
    w = 16
    while w >= 1:
        nc.vector.tensor_tensor(out=kf[:, 0:w, :], in0=kf[:, 0:w, :],
                                in1=kf[:, w:2 * w, :], op=A.add)
        w //= 2
    keys_bm = kf[:, 0, :]
    keysT = front.tile([64, 128], FP)                 # [(h e), ball]
    kt_ps = ps_tr.tile([64, 128], FP, tag="tr")
    nc.tensor.transpose(kt_ps, keys_bm, ident)
    nc.scalar.copy(out=keysT, in_=kt_ps)

    # ---------------- Stage C: own-point layouts --------------------------------
    # xr rows [0:1024] -> DRAM bounce -> point-major + transposed copies
    nc.sync.dma_start(out=xr_dram.ap().rearrange("(b m) d -> b m d", m=M),
                      in_=xr_bm[0:BPC, :, :])
    q_pm = big.tile([128, NT, H, EH], FP)  # per-partition q scalars
    nc.sync.dma_start(out=q_pm,
                      in_=xr_dram.ap().rearrange("(t p) (h e) -> p t h e", p=128, e=EH))
    x_own = big.tile([128, NT, D], FP)
    nc.sync.dma_start(out=x_own,
                      in_=x_d.ap()[0:NPC, :].rearrange("(t p) d -> p t d", p=128))

    qT = front.tile([64, NT, 128], FP)       # [(h e), nt, n128]
    for t in range(NT):
        q_ps = ps_tr.tile([64, 128], FP, tag="tr")
        nc.tensor.transpose(q_ps, q_pm[:, t].rearrange("p h e -> p (h e)"), ident)
        nc.scalar.copy(out=qT[:, t, :], in_=q_ps)

    # hi/lo bf16 split: 4-term split q.k = qhi.khi + qhi.klo + qlo.khi + qlo.klo
    # -> every bf16 product exact in fp32 -> logits match fp32 ref to ~2 ulp.
    # K-row order p = e*4 + j ; k terms [khi, klo, khi, klo], q [qhi, qhi, qlo, qlo]
    identB = consts.tile([128, 128], BF)
    nc.vector.tensor_copy(out=identB, in_=ident)
    kst4 = front.tile([64, 4, 128], BF)
    qst4 = front.tile([64, 4, NT, 128], BF)
    tmp32 = front.tile([64, NT, 128], FP, tag="tmp32")
    for (src_ap, dst, nfree) in ((keysT, kst4, 1), (qT, qst4, NT)):
        nc.vector.tensor_copy(out=dst[:, 0], in_=src_ap)     # hi (cast bf16)
        t32 = tmp32[:, 0:nfree, :] if nfree == NT else tmp32[:, 0, :]
        nc.vector.tensor_copy(out=t32, in_=dst[:, 0])        # hi back to fp32
        nc.vector.tensor_tensor(out=t32, in0=src_ap, in1=t32, op=A.subtract)
        nc.vector.tensor_copy(out=dst[:, 1], in_=t32)        # lo (cast bf16)
        if nfree == 1:   # k: [khi, klo, khi, klo]
            nc.vector.tensor_copy(out=dst[:, 2], in_=dst[:, 0])
            nc.vector.tensor_copy(out=dst[:, 3], in_=dst[:, 1])
        else:            # q: [qhi, qhi, qlo, qlo]
            nc.vector.tensor_copy(out=dst[:, 2], in_=dst[:, 1])
            nc.vector.tensor_copy(out=dst[:, 3], in_=dst[:, 1])
            nc.vector.tensor_copy(out=dst[:, 1], in_=dst[:, 0])
    kstack = big.tile([32, H, 128], BF)
    qstack = big.tile([32, H, NT, 128], BF)
    for h in range(H):
        # two DMA queues so the 16 strided loads overlap (head order kept)
        nc.sync.dma_start(
            out=kstack[:, h, :],
            in_=bass.AP(tensor=kst4.tensor, offset=kst4.offset + 8 * h * 4 * 128,
                        ap=[[4 * 128, 8], [128, 4], [1, 128]]))
        nc.gpsimd.dma_start(
            out=qstack[:, h],
            in_=bass.AP(tensor=qst4.tensor,
                        offset=qst4.offset + 8 * h * 4 * NT * 128,
                        ap=[[4 * NT * 128, 8], [NT * 128, 4], [128, NT],
                            [1, 128]]))
    # weights: w1/w2 [256, 64] -> transposed fp16 [64, 256]; w3 [64, 256] ->
    # [256, 64] (fp16: fp32 PE matmuls/transposes run at quarter rate)
    w1n = consts.tile([128, 2, D], FP)
    w2n = consts.tile([128, 2, D], FP)
    w3n = consts.tile([64, DH], FP)
    nc.sync.dma_start(out=w1n, in_=w1_d.ap().rearrange("(c j) d -> j c d", j=128))
    nc.sync.dma_start(out=w2n, in_=w2_d.ap().rearrange("(c j) d -> j c d", j=128))
    nc.sync.dma_start(out=w3n, in_=w3_d.ap())
    w1h = consts.tile([128, 2, D], F16)
    w2h = consts.tile([128, 2, D], F16)
    w3h = consts.tile([64, DH], F16)
    nc.vector.tensor_copy(out=w1h, in_=w1n)
    nc.vector.tensor_copy(out=w2h, in_=w2n)
    nc.vector.tensor_copy(out=w3h, in_=w3n)
    identH = consts.tile([128, 128], F16)
    nc.vector.tensor_copy(out=identH, in_=ident)
    w1T = consts.tile([64, DH], F16)   # [d, j]
    w2T = consts.tile([64, DH], F16)
    w3T = consts.tile([128, 2, D], F16)  # [j128, c, e]
    for c in range(2):
        t_ps = ps_tr.tile([64, 128], F16, tag="trh")
        nc.tensor.transpose(t_ps, w1h[:, c, :], identH)
        nc.scalar.copy(out=w1T[:, 128 * c:128 * (c + 1)], in_=t_ps)
        t_ps2 = ps_tr.tile([64, 128], F16, tag="trh")
        nc.tensor.transpose(t_ps2, w2h[:, c, :], identH)
        nc.scalar.copy(out=w2T[:, 128 * c:128 * (c + 1)], in_=t_ps2)
        t_ps3 = ps_tr.tile([128, 64], F16, tag="trh2")
        nc.tensor.transpose(t_ps3, w3h[:, 128 * c:128 * (c + 1)],
                            identH[0:64, 0:64])
        nc.scalar.copy(out=w3T[:, c, :], in_=t_ps3)
    b1_sb = consts.tile([128, 2], FP)
    b2_sb = consts.tile([128, 2], FP)
    nc.sync.dma_start(out=b1_sb, in_=b1_d.ap().rearrange("(c j) -> j c", j=128))
    nc.sync.dma_start(out=b2_sb, in_=b2_d.ap().rearrange("(c j) -> j c", j=128))
    b3_sb = consts.tile([128, D], FP)
    nc.sync.dma_start(out=b3_sb,
                      in_=bass.AP(tensor=b3_d, offset=0, ap=[[0, 128], [1, D]]))

    front_cm.__exit__(None, None, None)
    ps_tr_cm.__exit__(None, None, None)

    # ---------------- Stage D+E: selection + PE one-hot gather + attention -----
    # Per (h, t): PE logits -> DVE max8 -> DVE equality masks [n, b] (bf16)
    # -> PE transpose -> evac maskT -> PE gather matmuls -> evac G to bf16.
    # Then per-head batched DVE scores/softmax/weighted-sum (as v2).
    gpool = ctx.enter_context(tc.tile_pool(name="gpool", bufs=4))
    wpool = ctx.enter_context(tc.tile_pool(name="wpool", bufs=2))
    sel_cm = tc.tile_pool(name="sel", bufs=8)
    sel = sel_cm.__enter__()
    ps_lt_cm = tc.tile_pool(name="ps_lt", bufs=2, space="PSUM")
    ps_lt = ps_lt_cm.__enter__()
    ps_mt_cm = tc.tile_pool(name="ps_mt", bufs=2, space="PSUM")
    ps_mt = ps_mt_cm.__enter__()
    ps_g_cm = tc.tile_pool(name="ps_g", bufs=2, space="PSUM")
    ps_g = ps_g_cm.__enter__()

    attn16 = big.tile([128, NT, H, EH], BF)
    zh = big.tile([128, H, NT], FP)

    def selection(h, ts):
        """Logits (PE) + top-2 masks (DVE) for head h, tiles ts; returns masks.

        Split from the gather so its DVE ops can be emitted while head h-1's
        ACT exp runs, and the gather's ACT ops (maskT + evac) can be emitted
        AFTER head h-1's exp — engines are strict FIFO, so emission order on
        each engine is execution order."""
        masks = {}
        for i in range(0, len(ts), 2):
            ta, tb = ts[i], ts[i + 1]
            # two tiles' logits in one PSUM tile -> ONE is_equal for all 4
            # masks (the 120-cyc PSUM access penalty amortizes over 512 elems)
            lpm2 = ps_lt.tile([128, 2, 128], FP, tag="lt")
            nc.tensor.matmul(lpm2[:, 0], lhsT=qstack[:, h, ta, :],
                             rhs=kstack[:, h, :], start=True, stop=True)
            nc.tensor.matmul(lpm2[:, 1], lhsT=qstack[:, h, tb, :],
                             rhs=kstack[:, h, :], start=True, stop=True)
            v8p = sel.tile([128, 2, 8], FP, tag="v8")
            nc.vector.max(out=v8p[:, 0], in_=lpm2[:, 0])
            nc.vector.max(out=v8p[:, 1], in_=lpm2[:, 1])
            # mask4[p, pair, tk, b] = (lpm == v8[tk])
            mask4 = sel.tile([128, 2, TOPK, 128], BF, tag="mnb")
            nc.vector.tensor_tensor(
                out=mask4,
                in0=bass.AP(tensor=lpm2.tensor, offset=lpm2.offset,
                            ap=[lpm2.ap[0], [128, 2], [0, TOPK], [1, 128]]),
                in1=bass.AP(tensor=v8p.tensor, offset=v8p.offset,
                            ap=[v8p.ap[0], [8, 2], [1, TOPK], [0, 128]]),
                op=A.is_equal)
            masks[ta] = mask4[:, 0]
            masks[tb] = mask4[:, 1]
        return masks

    def gather_head(h, masks, ts, g_sb=None):
        """PE transposes + one-hot gathers + ACT evacs for head h, tiles ts.

        maskT(t+1) is emitted BEFORE evac(t) so the strict-FIFO ACT queue
        doesn't serialize ACT->PE->ACT per tile: gather(t) runs on PE while
        ACT does maskT(t+1), and evac(t) follows immediately."""
        if g_sb is None:
            g_sb = gpool.tile([128, NT, TOPK, M, EH], BF, tag="g")

        def emit_maskT_gather(t):
            mask2 = masks[t]
            mt_ps = ps_mt.tile([128, TOPK, 128], BF, tag="mt")
            for tk in range(TOPK):
                nc.tensor.transpose(mt_ps[:, tk, :], mask2[:, tk, :], identB)
            maskT = sel.tile([128, TOPK, 128], BF, tag="mT")
            nc.scalar.copy(out=maskT, in_=mt_ps)
            g_ps = ps_g.tile([128, TOPK, M, EH], FP, tag="g")
            for tk in range(TOPK):
                # rhs: xr16 [b, m, (h e)] -> per-head (m, e) column order
                nc.tensor.matmul(
                    g_ps[:, tk].rearrange("p m e -> p (m e)"),
                    lhsT=maskT[:, tk, :],
                    rhs=bass.AP(tensor=xr16.tensor,
                                offset=xr16.offset + EH * h,
                                ap=[xr16.ap[0], [D, M], [1, EH]]),
                    start=True, stop=True)
            return g_ps

        gp_prev, t_prev = emit_maskT_gather(ts[0]), ts[0]
        for t in ts[1:]:
            gp = emit_maskT_gather(t)
            # evac PSUM -> bf16 SBUF on ACT (DVE is the bottleneck engine)
            nc.scalar.copy(out=g_sb[:, t_prev], in_=gp_prev)
            gp_prev, t_prev = gp, t
        nc.scalar.copy(out=g_sb[:, t_prev], in_=gp_prev)
        return g_sb

    def compute_a(h, g_sb):
        """Scores + batched exp for head h (DVE prod/e-folds, ACT exp)."""
        nt = NT
        ng = nt * TOPK
        # bf16 q for this head, replicated over topk: [p, (t k), e]
        q2h = wpool.tile([128, nt, TOPK, EH], BF, tag="q2")
        nc.vector.tensor_copy(
            out=q2h,
            in_=bass.AP(tensor=q_pm.tensor,
                        offset=q_pm.offset + EH * h,
                        ap=[q_pm.ap[0], [H * EH, nt], [0, TOPK], [1, EH]]))
        g_v = g_sb.rearrange("p t k m e -> p (t k) m e")
        prod = wpool.tile([128, ng, M, EH], BF, tag="prod")
        q2_bc = bass.AP(tensor=q2h.tensor, offset=q2h.offset,
                        ap=[q2h.ap[0], [EH, ng], [0, M], [1, EH]])
        nc.vector.tensor_tensor(out=prod, in0=g_v, in1=q2_bc, op=A.mult)
        # s[p, (g m)] fp16 = sum_e prod, via 3 pair-folds over e (e pairs are
        # step-1 runs of 4/2/1 -> cheaper than the 1x-mode strided reduce).
        # scr16 is shared by s4 (e-folds) and p2k (m-folds) — disjoint lifetimes
        scr16 = wpool.tile([128, NT * M * EH], F16, tag="scr")
        s4 = scr16[:, 0:ng * M * 4].rearrange("p (j f) -> p j f", f=4)
        nc.vector.tensor_tensor(
            out=s4,
            in0=bass.AP(tensor=prod.tensor, offset=prod.offset,
                        ap=[prod.ap[0], [EH, ng * M], [1, 4]]),
            in1=bass.AP(tensor=prod.tensor, offset=prod.offset + 4,
                        ap=[prod.ap[0], [EH, ng * M], [1, 4]]),
            op=A.add)
        nc.vector.tensor_tensor(
            out=bass.AP(tensor=s4.tensor, offset=s4.offset,
                        ap=[s4.ap[0], [4, ng * M], [1, 2]]),
            in0=bass.AP(tensor=s4.tensor, offset=s4.offset,
                        ap=[s4.ap[0], [4, ng * M], [1, 2]]),
            in1=bass.AP(tensor=s4.tensor, offset=s4.offset + 2,
                        ap=[s4.ap[0], [4, ng * M], [1, 2]]),
            op=A.add)
        s_sb = wpool.tile([128, ng, M], F16, tag="s")
        nc.vector.tensor_tensor(
            out=s_sb,
            in0=bass.AP(tensor=s4.tensor, offset=s4.offset,
                        ap=[s4.ap[0], [4, ng * M]]),
            in1=bass.AP(tensor=s4.tensor, offset=s4.offset + 1,
                        ap=[s4.ap[0], [4, ng * M]]),
            op=A.add)
        # softmax numerator: p = exp(s/sqrt8), written REPLICATED over e by
        # re-reading s 8x (step-0 input dim) -> prod2 becomes a flat 2x TT.
        # Emitted in tile-halves so ACT exp overlaps DVE prod2 in compute_b.
        p_rep = wpool.tile([128, ng, M, EH], BF, tag="prep")
        hj = ng * M // 2
        for hf in range(2):
            sl = slice(hf * hj, (hf + 1) * hj)
            nc.scalar.activation(
                out=p_rep.rearrange("p g m e -> p (g m) e")[:, sl],
                in_=bass.AP(tensor=s_sb.tensor, offset=s_sb.offset + hf * hj,
                            ap=[s_sb.ap[0], [1, hj], [0, EH]]),
                func=AF.Exp, scale=ISQ8)
        return g_v, p_rep, scr16

    def compute_b(h, state, t0=0, t1=NT):
        """Weighted sum + z + (k,m)-folds for head h, tiles [t0:t1) (DVE)."""
        g_v, p_rep, scr16 = state
        nt = t1 - t0
        ng = nt * TOPK
        j0 = t0 * TOPK * M                 # (g m)-flat element base
        base = j0 * EH                     # fully-flat element base
        prod2 = wpool.tile([128, NT * TOPK, M, EH], BF, tag="prod")
        hj = ng * M // 2
        for hf in range(2):
            sl = slice(j0 + hf * hj, j0 + (hf + 1) * hj)
            nc.vector.tensor_tensor(
                out=prod2.rearrange("p g m e -> p (g m) e")[:, sl],
                in0=g_v.rearrange("p g m e -> p (g m) e")[:, sl],
                in1=p_rep.rearrange("p g m e -> p (g m) e")[:, sl],
                op=A.mult)
        # z[p, t] = sum over (tk, m) of p (read one e-lane of the replica)
        nc.vector.tensor_reduce(
            out=zh[:, h, t0:t1],
            in_=bass.AP(tensor=p_rep.tensor, offset=p_rep.offset + base,
                        ap=[p_rep.ap[0], [TOPK * M * EH, nt], [EH, TOPK * M]]),
            axis=AX.X, op=A.add)
        # fold the two topk slots with a packed TT-add, then reduce over m by
        # halving folds on contiguous [m, e] runs (2x-eligible, vs 1x reduce)
        p2k = scr16[:, t0 * M * EH:t1 * M * EH].rearrange(
            "p (t m e) -> p t m e", m=M, e=EH)
        nc.vector.tensor_tensor(
            out=p2k,
            in0=bass.AP(tensor=prod2.tensor, offset=prod2.offset + base,
                        ap=[prod2.ap[0], [TOPK * M * EH, nt], [EH, M], [1, EH]]),
            in1=bass.AP(tensor=prod2.tensor,
                        offset=prod2.offset + base + M * EH,
                        ap=[prod2.ap[0], [TOPK * M * EH, nt], [EH, M], [1, EH]]),
            op=A.add)
        w = (M // 2) * EH
        while w > EH:
            nc.vector.tensor_tensor(
                out=bass.AP(tensor=p2k.tensor, offset=p2k.offset,
                            ap=[p2k.ap[0], [M * EH, nt], [1, w]]),
                in0=bass.AP(tensor=p2k.tensor, offset=p2k.offset,
                            ap=[p2k.ap[0], [M * EH, nt], [1, w]]),
                in1=bass.AP(tensor=p2k.tensor, offset=p2k.offset + w,
                            ap=[p2k.ap[0], [M * EH, nt], [1, w]]),
                op=A.add)
            w //= 2
        nc.vector.tensor_tensor(
            out=attn16[:, t0:t1, h, :],
            in0=bass.AP(tensor=p2k.tensor, offset=p2k.offset,
                        ap=[p2k.ap[0], [M * EH, nt], [1, EH]]),
            in1=bass.AP(tensor=p2k.tensor, offset=p2k.offset + EH,
                        ap=[p2k.ap[0], [M * EH, nt], [1, EH]]),
            op=A.add)

    # finalize tiles live in `big` so the per-half epilogue can interleave
    # with head 7's split compute (attention pools still open)
    zinv = big.tile([128, H, NT], FP)
    attn = big.tile([128, NT, D], FP)
    y = big.tile([128, NT, D], FP)
    sq2 = big.tile([128, NT, D], FP)
    ri2 = big.tile([128, NT], FP)
    ln2 = big.tile([128, NT], FP)
    x2 = big.tile([128, NT, D], F16)
    n2w_sb = consts.tile([128, D], FP)
    nc.sync.dma_start(out=n2w_sb,
                      in_=bass.AP(tensor=n2w_d, offset=0, ap=[[0, 128], [1, D]]))

    def finalize_a(t0, t1):
        """attn normalize + residual + rmsnorm2 + x2 for tiles [t0:t1)."""
        nt = t1 - t0
        nc.vector.reciprocal(out=zinv[:, :, t0:t1], in_=zh[:, :, t0:t1])
        zinv_bc = bass.AP(tensor=zinv.tensor, offset=zinv.offset + t0,
                          ap=[zinv.ap[0], [1, nt], [NT, H], [0, EH]])
        nc.vector.tensor_tensor(
            out=attn.rearrange("p t (h e) -> p t h e", e=EH)[:, t0:t1],
            in0=attn16[:, t0:t1], in1=zinv_bc, op=A.mult)
        nc.vector.tensor_tensor(out=y[:, t0:t1], in0=x_own[:, t0:t1],
                                in1=attn[:, t0:t1], op=A.add)
        nc.scalar.activation(out=sq2[:, t0:t1], in_=y[:, t0:t1], func=AF.Square)
        nc.vector.tensor_reduce(out=ri2[:, t0:t1], in_=sq2[:, t0:t1],
                                axis=AX.X, op=A.add)
        nc.vector.tensor_scalar(ri2[:, t0:t1], ri2[:, t0:t1], 1.0 / D, EPS,
                                op0=A.mult, op1=A.add)
        nc.scalar.activation(out=ln2[:, t0:t1], in_=ri2[:, t0:t1], func=AF.Ln)
        nc.scalar.activation(out=ri2[:, t0:t1], in_=ln2[:, t0:t1],
                             func=AF.Exp, scale=-0.5)
        nc.vector.tensor_tensor(out=x2[:, t0:t1], in0=y[:, t0:t1],
                                in1=_bc(ri2[:, t0:t1], 2, D), op=A.mult)
        nc.vector.tensor_tensor(out=x2[:, t0:t1], in0=x2[:, t0:t1],
                                in1=_bc(n2w_sb, 1, nt), op=A.mult)

    # Software pipeline over heads. Emission order IS execution order on each
    # strict-FIFO engine, so head h+1's selection/gather is emitted in two
    # tile-halves AROUND head h's exp: ACT runs [mT/evac(h+1) 0-3, exp(h),
    # mT/evac(h+1) 4-7] while DVE runs [masks(h+1) 0-3, prod(h), e-folds(h),
    # masks(h+1) 4-7, prod2(h), folds(h)] with no exp-wait bubble.
    half0, half1 = list(range(NT // 2)), list(range(NT // 2, NT))
    g_cur = gather_head(0, selection(0, half0 + half1), half0 + half1)
    for h in range(H - 1):
        m0 = selection(h + 1, half0)
        g_next = gather_head(h + 1, m0, half0)
        st = compute_a(h, g_cur)
        m1 = selection(h + 1, half1)
        gather_head(h + 1, m1, half1, g_sb=g_next)
        compute_b(h, st)
        g_cur = g_next
    st_last = compute_a(H - 1, g_cur)
    ps_g_cm.__exit__(None, None, None)
    ps_mt_cm.__exit__(None, None, None)
    ps_lt_cm.__exit__(None, None, None)
    sel_cm.__exit__(None, None, None)

    mlpw = ctx.enter_context(tc.tile_pool(name="mlpw", bufs=1))
    ps_m = ctx.enter_context(tc.tile_pool(name="ps_m", bufs=1, space="PSUM"))
    x2T = mlpw.tile([64, NT, 128], F16)
    hhT = mlpw.tile([128, 2, NT, 128], F16)   # [j128, c, nt, n]
    final = mlpw.tile([128, NT, D], FP)

    def mlp_half(t0, t1):
        """x2T transposes + SwiGLU MLP (fp16) + residual + out DMA for [t0:t1)."""
        nt = t1 - t0
        for t in range(t0, t1):
            xt_ps = ps_m.tile([64, 128], F16, tag="tr")
            nc.tensor.transpose(xt_ps, x2[:, t, :], identH)
            nc.scalar.copy(out=x2T[:, t, :], in_=xt_ps)
        nc.vector.tensor_tensor(out=y[:, t0:t1], in0=y[:, t0:t1],
                                in1=_bc(b3_sb, 1, nt), op=A.add)
        for c in range(2):
            h1_ps = ps_m.tile([128, nt, 128], FP, tag="h1")
            h2_ps = ps_m.tile([128, nt, 128], FP, tag="h2")
            nc.tensor.matmul(h1_ps.rearrange("j t n -> j (t n)"),
                             lhsT=w1T[:, 128 * c:128 * (c + 1)],
                             rhs=x2T[:, t0:t1].rearrange("d t n -> d (t n)"),
                             start=True, stop=True)
            nc.tensor.matmul(h2_ps.rearrange("j t n -> j (t n)"),
                             lhsT=w2T[:, 128 * c:128 * (c + 1)],
                             rhs=x2T[:, t0:t1].rearrange("d t n -> d (t n)"),
                             start=True, stop=True)
            h1b = mlpw.tile([128, nt, 128], F16, tag="h1b")
            nc.vector.tensor_scalar(h1b, h1_ps, b1_sb[:, c:c + 1], None,
                                    op0=A.add)
            sgm = mlpw.tile([128, nt, 128], F16, tag="sgm")
            nc.scalar.activation(out=sgm, in_=h1b, func=AF.Silu)
            h2s = mlpw.tile([128, nt, 128], F16, tag="h2s")
            nc.vector.tensor_scalar(h2s, h2_ps, b2_sb[:, c:c + 1], None,
                                    op0=A.add)
            nc.vector.tensor_tensor(out=hhT[:, c, t0:t1], in0=sgm, in1=h2s,
                                    op=A.mult)
        for t in range(t0, t1):
            o_ps = ps_m.tile([128, D], FP, tag="o")
            for c in range(2):
                nc.tensor.matmul(o_ps, lhsT=hhT[:, c, t, :], rhs=w3T[:, c, :],
                                 start=(c == 0), stop=(c == 1))
            nc.vector.scalar_tensor_tensor(out=final[:, t], in0=o_ps,
                                           scalar=1.0, in1=y[:, t],
                                           op0=A.mult, op1=A.add)
        nc.sync.dma_start(
            out=out_d.ap().rearrange("(t p) d -> p t d", p=128)[:, t0:t1],
            in_=final[:, t0:t1])

    # Split epilogue: head 7's second-half folds overlap finalize of the
    # first half; the MLP halves pipeline behind finalize.
    compute_b(H - 1, st_last, 0, NT // 2)
    finalize_a(0, NT // 2)
    compute_b(H - 1, st_last, NT // 2, NT)
    finalize_a(NT // 2, NT)
    mlp_half(0, NT // 2)
    mlp_half(NT // 2, NT)


def build_program():
    from contextlib import ExitStack
    nc = bacc.Bacc("TRN2", target_bir_lowering=False, debug=False,
                   num_devices=NCORES)
    x_d = nc.dram_tensor("x", [N, D], FP, kind="ExternalInput")
    pos_d = nc.dram_tensor("pos", [N, D], FP, kind="ExternalInput")
    n1w_d = nc.dram_tensor("norm1_w", [D], FP, kind="ExternalInput")
    n2w_d = nc.dram_tensor("norm2_w", [D], FP, kind="ExternalInput")
    w1_d = nc.dram_tensor("w1_w", [DH, D], FP, kind="ExternalInput")
    b1_d = nc.dram_tensor("w1_b", [DH], FP, kind="ExternalInput")
    w2_d = nc.dram_tensor("w2_w", [DH, D], FP, kind="ExternalInput")
    b2_d = nc.dram_tensor("w2_b", [DH], FP, kind="ExternalInput")
    w3_d = nc.dram_tensor("w3_w", [D, DH], FP, kind="ExternalInput")
    b3_d = nc.dram_tensor("w3_b", [D], FP, kind="ExternalInput")
    out_d = nc.dram_tensor("out", [NPC, D], FP, kind="ExternalOutput")
    xr_dram = nc.dram_tensor("xr_own", [NPC, D], FP)

    tensors = (x_d, pos_d, n1w_d, n2w_d, w1_d, b1_d, w2_d, b2_d, w3_d, b3_d,
               out_d, xr_dram)
    with tile.TileContext(nc) as tc:
        with ExitStack() as ctx:
            build_kernel_body(nc, tc, ctx, tensors)
    nc.compile()
    return nc


_NC_CACHE = None


def _get_nc():
    global _NC_CACHE
    if _NC_CACHE is None:
        _NC_CACHE = build_program()
    return _NC_CACHE


def make_in_maps(inputs):
    in_maps = []
    for c in range(NCORES):
        sh = c * NPC
        m = {
            "x": np.ascontiguousarray(np.roll(inputs["x"], -sh, axis=0)),
            "pos": np.ascontiguousarray(np.roll(inputs["pos"], -sh, axis=0)),
            "norm1_w": inputs["norm1_w"], "norm2_w": inputs["norm2_w"],
            "w1_w": inputs["w1_w"], "w1_b": inputs["w1_b"],
            "w2_w": inputs["w2_w"], "w2_b": inputs["w2_b"],
            "w3_w": inputs["w3_w"], "w3_b": inputs["w3_b"],
        }
        in_maps.append({k: np.asarray(v, dtype=np.float32) for k, v in m.items()})
    return in_maps


def run_on_hw(inputs, trace=False):
    from concourse.bass_utils import run_bass_kernel_spmd
    nc = _get_nc()
    res = run_bass_kernel_spmd(nc, make_in_maps(inputs), list(range(NCORES)),
                               trace=trace)
    out = np.concatenate([res.results[c]["out"] for c in range(NCORES)], axis=0)
    return out, res


def kernel(**inputs):
    out, _ = run_on_hw(inputs, trace=False)
    return out



# revision 59
# speedup vs baseline: 1.0305x; 1.0080x over previous
"""Trainium2 Bass kernel for nn_BasicLayer (Erwin NSA-MSA sparse ball attention).

8 NeuronCores, data-parallel over points: each core receives the full x/pos
ROTATED so its own 1024 points sit at rows [0:1024] (whole-ball rotation makes
one SPMD program correct for every core; no collectives).

Per core:
  - Stage A: xr = rmsnorm(x)*n1w + rel for all 8192 points, ball-per-partition;
    bf16 copy of xr kept as the gather source; ball-summary keys by reduction.
    Ball pos-means via flat halving folds; rsqrt = exp(-0.5*ln(v)) + Newton
    (keeps ACT in the natural_log_exp table set used by attention exp).
  - Routing logits via 4-term bf16-split PE matmuls (bit-exact vs fp32 ref, so
    top-2 ball selection matches jax.lax.top_k except true fp32 ties).
  - Per (head, tile): DVE max8 -> equality masks [n,b] in bf16 -> PE transpose
    -> PE one-hot gather matmuls -> single PSUM->SBUF bf16 evac (ACT).
  - Per head, batched across tiles (DVE tensor_reduce runs at 1x, so all
    reductions are pairwise halving folds that hit the 2x bf16/fp16 TT mode):
    scores product (2x: e-innermost broadcast AP packs) + 3 e-pair-folds;
    exp emitted e-REPLICATED by ACT (step-0 input re-read) so the
    weighted-sum product is a flat 2x TT, in tile-halves for ACT/DVE overlap;
    z by strided reduce of one e-lane; (k,m)-reduce via contiguous [m,e]-run
    halving folds in fp16.
  - Residual + RMSNorm + SwiGLU MLP (fp16 weights/transposes/matmuls on PE --
    fp32 PE runs at quarter rate; Silu in one ACT op, emitted last so its
    table set loads once).
"""

import numpy as np

import concourse.bacc as bacc
import concourse.bass as bass
import concourse.mybir as mybir
import concourse.tile as tile
from concourse.masks import make_identity

FP = mybir.dt.float32
BF = mybir.dt.bfloat16
F16 = mybir.dt.float16
U16 = mybir.dt.uint16
I16 = mybir.dt.int16

N, D = 8192, 64
M = 64          # ball size
NB = N // M     # 128 balls
H, EH = 8, 8
TOPK = 2
NCORES = 8
NPC = N // NCORES   # 1024 points per core
NT = NPC // 128     # 8 point-tiles of 128
BPC = NPC // M      # 16 own balls per core
DH = D * 4          # 256 mlp hidden
EPS = 1.1920929e-07
ISQ8 = float(1.0 / np.sqrt(EH))
EM = M * EH         # 512 = gathered elem size (m-major, e innermost)
NG = NT * TOPK      # 16 gather slots per point
NIDX = NG * 128     # 2048 gathered blocks per head

A = mybir.AluOpType
AF = mybir.ActivationFunctionType
AX = mybir.AxisListType


def _bc(ap, dim, count):
    """Insert a step-0 (broadcast) dim at position `dim` of an AP."""
    new = [list(p) for p in ap.ap]
    new.insert(dim, [0, count])
    return bass.AP(tensor=ap.tensor, offset=ap.offset, ap=new)


def build_kernel_body(nc, tc, ctx, tensors):
    (x_d, pos_d, n1w_d, n2w_d, w1_d, b1_d, w2_d, b2_d, w3_d, b3_d,
     out_d, xr_dram) = tensors

    consts = ctx.enter_context(tc.tile_pool(name="consts", bufs=1))
    big = ctx.enter_context(tc.tile_pool(name="big", bufs=1))
    front_cm = tc.tile_pool(name="front", bufs=1)
    front = front_cm.__enter__()
    ps_tr_cm = tc.tile_pool(name="ps_tr", bufs=2, space="PSUM")
    ps_tr = ps_tr_cm.__enter__()

    ident = consts.tile([128, 128], FP)
    make_identity(nc, ident)


    # ---------------- Stage A: load + xr = rmsnorm(x)*n1w + rel (ball-major) ----
    x_bm = front.tile([128, M, D], FP)       # [ball, m, d]
    pos_bm = front.tile([128, M, D], FP)
    # x and pos on different DMA queues so the two 2MB loads overlap
    nc.sync.dma_start(out=x_bm, in_=x_d.ap().rearrange("(b m) d -> b m d", m=M))
    nc.gpsimd.dma_start(out=pos_bm,
                        in_=pos_d.ap().rearrange("(b m) d -> b m d", m=M))

    n1w_sb = consts.tile([128, D], FP)
    nc.sync.dma_start(out=n1w_sb,
                      in_=bass.AP(tensor=n1w_d, offset=0, ap=[[0, 128], [1, D]]))

    # ball mean of pos (over m): flat contiguous halving folds (m-major)
    mpf = front.tile([128, 32, D], FP, tag="mpf")
    nc.vector.tensor_tensor(out=mpf, in0=pos_bm[:, 0:32, :],
                            in1=pos_bm[:, 32:64, :], op=A.add)
    w = 16
    while w >= 1:
        nc.vector.tensor_tensor(out=mpf[:, 0:w, :], in0=mpf[:, 0:w, :],
                                in1=mpf[:, w:2 * w, :], op=A.add)
        w //= 2
    mp = front.tile([128, D], FP, tag="mp")
    nc.vector.tensor_scalar(mp, mpf[:, 0, :], 1.0 / M, None, op0=A.mult)

    # rms: 1/sqrt(mean(x^2) + eps)
    sq = front.tile([128, M, D], FP, tag="sq")
    nc.scalar.activation(out=sq, in_=x_bm, func=AF.Square)
    sq8 = front.tile([128, M, 8], FP, tag="sq8")
    nc.vector.tensor_reduce(out=sq8, in_=sq.rearrange("b m (g d) -> b m g d", g=8),
                            axis=AX.X, op=A.add)
    msq = front.tile([128, M], FP, tag="msq")
    nc.vector.tensor_reduce(out=msq, in_=sq8, axis=AX.X, op=A.add)
    nc.vector.tensor_scalar(msq, msq, 1.0 / D, EPS, op0=A.mult, op1=A.add)
    rinv = front.tile([128, M], FP, tag="rinv")
    lnv = front.tile([128, M], FP, tag="lnv")
    nc.scalar.activation(out=lnv, in_=msq, func=AF.Ln)
    nc.scalar.activation(out=rinv, in_=lnv, func=AF.Exp, scale=-0.5)
    # one Newton step: r <- r*(1.5 - 0.5*msq*r^2)
    rsqv = front.tile([128, M], FP, tag="rsqv")
    nc.vector.tensor_tensor(out=rsqv, in0=rinv, in1=rinv, op=A.mult)
    nc.vector.tensor_tensor(out=rsqv, in0=rsqv, in1=msq, op=A.mult)
    nc.vector.tensor_scalar(rsqv, rsqv, -0.5, 1.5, op0=A.mult, op1=A.add)
    nc.vector.tensor_tensor(out=rinv, in0=rinv, in1=rsqv, op=A.mult)

    nc.vector.tensor_tensor(out=pos_bm, in0=pos_bm, in1=_bc(mp, 1, M),
                            op=A.subtract)      # pos_bm becomes rel
    xr_bm = front.tile([128, M, D], FP)
    nc.vector.tensor_tensor(out=xr_bm, in0=x_bm, in1=_bc(rinv, 2, D), op=A.mult)
    nc.vector.tensor_tensor(out=xr_bm, in0=xr_bm, in1=_bc(n1w_sb, 1, M), op=A.mult)
    nc.vector.tensor_tensor(out=xr_bm, in0=xr_bm, in1=pos_bm, op=A.add)

    # ---------------- Stage B: KB to DRAM (bf16) + ball-summary keys ----------
    xr16 = big.tile([128, M, D], BF)
    nc.vector.tensor_copy(out=xr16, in_=xr_bm)

    # ball-summary keys: flat halving folds over m (strided reduce is 2x slower)
    kf = front.tile([128, 32, D], FP, tag="kf")
    nc.vector.tensor_tensor(out=kf, in0=xr_bm[:, 0:32, :],
                            in1=xr_bm[:, 32:64, :], op=A.add)
    w = 16
    while w >= 1:
        nc.vector.tensor_tensor(out=kf[:, 0:w, :], in0=kf[:, 0:w, :],
                                in1=kf[:, w:2 * w, :], op=A.add)
        w //= 2
    keys_bm = kf[:, 0, :]
    keysT = front.tile([64, 128], FP)                 # [(h e), ball]
    kt_ps = ps_tr.tile([64, 128], FP, tag="tr")
    nc.tensor.transpose(kt_ps, keys_bm, ident)
    nc.scalar.copy(out=keysT, in_=kt_ps)

    # ---------------- Stage C: own-point layouts --------------------------------
    # xr rows [0:1024] -> DRAM bounce -> point-major + transposed copies
    nc.sync.dma_start(out=xr_dram.ap().rearrange("(b m) d -> b m d", m=M),
                      in_=xr_bm[0:BPC, :, :])
    q_pm = big.tile([128, NT, H, EH], FP)  # per-partition q scalars
    nc.sync.dma_start(out=q_pm,
                      in_=xr_dram.ap().rearrange("(t p) (h e) -> p t h e", p=128, e=EH))
    x_own = big.tile([128, NT, D], FP)
    nc.sync.dma_start(out=x_own,
                      in_=x_d.ap()[0:NPC, :].rearrange("(t p) d -> p t d", p=128))

    qT = front.tile([64, NT, 128], FP)       # [(h e), nt, n128]
    for t in range(NT):
        q_ps = ps_tr.tile([64, 128], FP, tag="tr")
        nc.tensor.transpose(q_ps, q_pm[:, t].rearrange("p h e -> p (h e)"), ident)
        nc.scalar.copy(out=qT[:, t, :], in_=q_ps)

    # hi/lo bf16 split: 4-term split q.k = qhi.khi + qhi.klo + qlo.khi + qlo.klo
    # -> every bf16 product exact in fp32 -> logits match fp32 ref to ~2 ulp.
    # K-row order p = e*4 + j ; k terms [khi, klo, khi, klo], q [qhi, qhi, qlo, qlo]
    identB = consts.tile([128, 128], BF)
    nc.vector.tensor_copy(out=identB, in_=ident)
    kst4 = front.tile([64, 4, 128], BF)
    qst4 = front.tile([64, 4, NT, 128], BF)
    tmp32 = front.tile([64, NT, 128], FP, tag="tmp32")
    for (src_ap, dst, nfree) in ((keysT, kst4, 1), (qT, qst4, NT)):
        nc.vector.tensor_copy(out=dst[:, 0], in_=src_ap)     # hi (cast bf16)
        t32 = tmp32[:, 0:nfree, :] if nfree == NT else tmp32[:, 0, :]
        nc.vector.tensor_copy(out=t32, in_=dst[:, 0])        # hi back to fp32
        nc.vector.tensor_tensor(out=t32, in0=src_ap, in1=t32, op=A.subtract)
        nc.vector.tensor_copy(out=dst[:, 1], in_=t32)        # lo (cast bf16)
        if nfree == 1:   # k: [khi, klo, khi, klo]
            nc.vector.tensor_copy(out=dst[:, 2], in_=dst[:, 0])
            nc.vector.tensor_copy(out=dst[:, 3], in_=dst[:, 1])
        else:            # q: [qhi, qhi, qlo, qlo]
            nc.vector.tensor_copy(out=dst[:, 2], in_=dst[:, 1])
            nc.vector.tensor_copy(out=dst[:, 3], in_=dst[:, 1])
            nc.vector.tensor_copy(out=dst[:, 1], in_=dst[:, 0])
    kstack = big.tile([32, H, 128], BF)
    qstack = big.tile([32, H, NT, 128], BF)
    for h in range(H):
        # two DMA queues so the 16 strided loads overlap (head order kept)
        nc.sync.dma_start(
            out=kstack[:, h, :],
            in_=bass.AP(tensor=kst4.tensor, offset=kst4.offset + 8 * h * 4 * 128,
                        ap=[[4 * 128, 8], [128, 4], [1, 128]]))
        nc.gpsimd.dma_start(
            out=qstack[:, h],
            in_=bass.AP(tensor=qst4.tensor,
                        offset=qst4.offset + 8 * h * 4 * NT * 128,
                        ap=[[4 * NT * 128, 8], [NT * 128, 4], [128, NT],
                            [1, 128]]))
    # weights: w1/w2 [256, 64] -> transposed fp16 [64, 256]; w3 [64, 256] ->
    # [256, 64] (fp16: fp32 PE matmuls/transposes run at quarter rate)
    w1n = consts.tile([128, 2, D], FP)
    w2n = consts.tile([128, 2, D], FP)
    w3n = consts.tile([64, DH], FP)
    nc.sync.dma_start(out=w1n, in_=w1_d.ap().rearrange("(c j) d -> j c d", j=128))
    nc.sync.dma_start(out=w2n, in_=w2_d.ap().rearrange("(c j) d -> j c d", j=128))
    nc.sync.dma_start(out=w3n, in_=w3_d.ap())
    w1h = consts.tile([128, 2, D], F16)
    w2h = consts.tile([128, 2, D], F16)
    w3h = consts.tile([64, DH], F16)
    nc.vector.tensor_copy(out=w1h, in_=w1n)
    nc.vector.tensor_copy(out=w2h, in_=w2n)
    nc.vector.tensor_copy(out=w3h, in_=w3n)
    identH = consts.tile([128, 128], F16)
    nc.vector.tensor_copy(out=identH, in_=ident)
    w1T = consts.tile([64, DH], F16)   # [d, j]
    w2T = consts.tile([64, DH], F16)
    w3T = consts.tile([128, 2, D], F16)  # [j128, c, e]
    for c in range(2):
        t_ps = ps_tr.tile([64, 128], F16, tag="trh")
        nc.tensor.transpose(t_ps, w1h[:, c, :], identH)
        nc.scalar.copy(out=w1T[:, 128 * c:128 * (c + 1)], in_=t_ps)
        t_ps2 = ps_tr.tile([64, 128], F16, tag="trh")
        nc.tensor.transpose(t_ps2, w2h[:, c, :], identH)
        nc.scalar.copy(out=w2T[:, 128 * c:128 * (c + 1)], in_=t_ps2)
        t_ps3 = ps_tr.tile([128, 64], F16, tag="trh2")
        nc.tensor.transpose(t_ps3, w3h[:, 128 * c:128 * (c + 1)],
                            identH[0:64, 0:64])
        nc.scalar.copy(out=w3T[:, c, :], in_=t_ps3)
    b1_sb = consts.tile([128, 2], FP)
    b2_sb = consts.tile([128, 2], FP)
    nc.sync.dma_start(out=b1_sb, in_=b1_d.ap().rearrange("(c j) -> j c", j=128))
    nc.sync.dma_start(out=b2_sb, in_=b2_d.ap().rearrange("(c j) -> j c", j=128))
    b3_sb = consts.tile([128, D], FP)
    nc.sync.dma_start(out=b3_sb,
                      in_=bass.AP(tensor=b3_d, offset=0, ap=[[0, 128], [1, D]]))

    front_cm.__exit__(None, None, None)
    ps_tr_cm.__exit__(None, None, None)

    # ---------------- Stage D+E: selection + PE one-hot gather + attention -----
    # Per (h, t): PE logits -> DVE max8 -> DVE equality masks [n, b] (bf16)
    # -> PE transpose -> evac maskT -> PE gather matmuls -> evac G to bf16.
    # Then per-head batched DVE scores/softmax/weighted-sum (as v2).
    gpool = ctx.enter_context(tc.tile_pool(name="gpool", bufs=4))
    wpool = ctx.enter_context(tc.tile_pool(name="wpool", bufs=2))
    sel_cm = tc.tile_pool(name="sel", bufs=4)   # pair-batched: <=4 live/tag
    sel = sel_cm.__enter__()
    ps_lt_cm = tc.tile_pool(name="ps_lt", bufs=2, space="PSUM")
    ps_lt = ps_lt_cm.__enter__()
    ps_mt_cm = tc.tile_pool(name="ps_mt", bufs=2, space="PSUM")
    ps_mt = ps_mt_cm.__enter__()
    ps_g_cm = tc.tile_pool(name="ps_g", bufs=2, space="PSUM")
    ps_g = ps_g_cm.__enter__()

    attn16 = big.tile([128, NT, H, EH], BF)
    zh = big.tile([128, H, NT], FP)

    def selection(h, ts):
        """Logits (PE) + top-2 masks (DVE) for head h, tiles ts; returns masks.

        Split from the gather so its DVE ops can be emitted while head h-1's
        ACT exp runs, and the gather's ACT ops (maskT + evac) can be emitted
        AFTER head h-1's exp — engines are strict FIFO, so emission order on
        each engine is execution order."""
        masks = {}
        for i in range(0, len(ts), 2):
            ta, tb = ts[i], ts[i + 1]
            # two tiles' logits in one PSUM tile -> ONE is_equal for all 4
            # masks (the 120-cyc PSUM access penalty amortizes over 512 elems)
            lpm2 = ps_lt.tile([128, 2, 128], FP, tag="lt")
            nc.tensor.matmul(lpm2[:, 0], lhsT=qstack[:, h, ta, :],
                             rhs=kstack[:, h, :], start=True, stop=True)
            nc.tensor.matmul(lpm2[:, 1], lhsT=qstack[:, h, tb, :],
                             rhs=kstack[:, h, :], start=True, stop=True)
            v8p = sel.tile([128, 2, 8], FP, tag="v8")
            nc.vector.max(out=v8p[:, 0], in_=lpm2[:, 0])
            nc.vector.max(out=v8p[:, 1], in_=lpm2[:, 1])
            # mask4[p, pair, tk, b] = (lpm == v8[tk])
            mask4 = sel.tile([128, 2, TOPK, 128], BF, tag="mnb")
            nc.vector.tensor_tensor(
                out=mask4,
                in0=bass.AP(tensor=lpm2.tensor, offset=lpm2.offset,
                            ap=[lpm2.ap[0], [128, 2], [0, TOPK], [1, 128]]),
                in1=bass.AP(tensor=v8p.tensor, offset=v8p.offset,
                            ap=[v8p.ap[0], [8, 2], [1, TOPK], [0, 128]]),
                op=A.is_equal)
            masks[ta] = mask4[:, 0]
            masks[tb] = mask4[:, 1]
        return masks

    def gather_head(h, masks, ts, g_sb=None):
        """PE transposes + one-hot gathers + ACT evacs for head h, tiles ts.

        maskT(t+1) is emitted BEFORE evac(t) so the strict-FIFO ACT queue
        doesn't serialize ACT->PE->ACT per tile: gather(t) runs on PE while
        ACT does maskT(t+1), and evac(t) follows immediately."""
        if g_sb is None:
            g_sb = gpool.tile([128, NT, TOPK, M, EH], BF, tag="g")

        def emit_maskT(pair):
            # 4 transposes of a tile-pair into ONE PSUM tile -> one ACT evac
            # (the 352-cyc ACT op overhead amortizes over 512 elems)
            mt_ps = ps_mt.tile([128, 2, TOPK, 128], BF, tag="mt")
            for j, t in enumerate(pair):
                for tk in range(TOPK):
                    nc.tensor.transpose(mt_ps[:, j, tk, :],
                                        masks[t][:, tk, :], identB)
            maskT4 = sel.tile([128, 2, TOPK, 128], BF, tag="mT")
            nc.scalar.copy(out=maskT4, in_=mt_ps)
            return maskT4

        def emit_gathers(pair, maskT4):
            gps = []
            for j, t in enumerate(pair):
                g_ps = ps_g.tile([128, TOPK, M, EH], FP, tag="g")
                for tk in range(TOPK):
                    # rhs: xr16 [b, m, (h e)] -> per-head (m, e) column order
                    nc.tensor.matmul(
                        g_ps[:, tk].rearrange("p m e -> p (m e)"),
                        lhsT=maskT4[:, j, tk, :],
                        rhs=bass.AP(tensor=xr16.tensor,
                                    offset=xr16.offset + EH * h,
                                    ap=[xr16.ap[0], [D, M], [1, EH]]),
                        start=True, stop=True)
                gps.append((t, g_ps))
            return gps

        pairs = [(ts[i], ts[i + 1]) for i in range(0, len(ts), 2)]
        gp_prev = emit_gathers(pairs[0], emit_maskT(pairs[0]))
        for pr in pairs[1:]:
            mT = emit_maskT(pr)
            # evac PSUM -> bf16 SBUF on ACT (DVE is the bottleneck engine);
            # emitted after the next pair's maskT so ACT never waits on PE
            for t, gp in gp_prev:
                nc.scalar.copy(out=g_sb[:, t], in_=gp)
            gp_prev = emit_gathers(pr, mT)
        for t, gp in gp_prev:
            nc.scalar.copy(out=g_sb[:, t], in_=gp)
        return g_sb

    def compute_a(h, g_sb):
        """Scores + batched exp for head h (DVE prod/e-folds, ACT exp)."""
        nt = NT
        ng = nt * TOPK
        # bf16 q for this head, replicated over topk: [p, (t k), e]
        q2h = wpool.tile([128, nt, TOPK, EH], BF, tag="q2")
        nc.vector.tensor_copy(
            out=q2h,
            in_=bass.AP(tensor=q_pm.tensor,
                        offset=q_pm.offset + EH * h,
                        ap=[q_pm.ap[0], [H * EH, nt], [0, TOPK], [1, EH]]))
        g_v = g_sb.rearrange("p t k m e -> p (t k) m e")
        prod = wpool.tile([128, ng, M, EH], BF, tag="prod")
        q2_bc = bass.AP(tensor=q2h.tensor, offset=q2h.offset,
                        ap=[q2h.ap[0], [EH, ng], [0, M], [1, EH]])
        nc.vector.tensor_tensor(out=prod, in0=g_v, in1=q2_bc, op=A.mult)
        # s[p, (g m)] fp16 = sum_e prod, via 3 pair-folds over e (e pairs are
        # step-1 runs of 4/2/1 -> cheaper than the 1x-mode strided reduce).
        # scr16 is shared by s4 (e-folds) and p2k (m-folds) — disjoint lifetimes
        scr16 = wpool.tile([128, NT * M * EH], F16, tag="scr")
        s4 = scr16[:, 0:ng * M * 4].rearrange("p (j f) -> p j f", f=4)
        nc.vector.tensor_tensor(
            out=s4,
            in0=bass.AP(tensor=prod.tensor, offset=prod.offset,
                        ap=[prod.ap[0], [EH, ng * M], [1, 4]]),
            in1=bass.AP(tensor=prod.tensor, offset=prod.offset + 4,
                        ap=[prod.ap[0], [EH, ng * M], [1, 4]]),
            op=A.add)
        nc.vector.tensor_tensor(
            out=bass.AP(tensor=s4.tensor, offset=s4.offset,
                        ap=[s4.ap[0], [4, ng * M], [1, 2]]),
            in0=bass.AP(tensor=s4.tensor, offset=s4.offset,
                        ap=[s4.ap[0], [4, ng * M], [1, 2]]),
            in1=bass.AP(tensor=s4.tensor, offset=s4.offset + 2,
                        ap=[s4.ap[0], [4, ng * M], [1, 2]]),
            op=A.add)
        s_sb = wpool.tile([128, ng, M], F16, tag="s")
        nc.vector.tensor_tensor(
            out=s_sb,
            in0=bass.AP(tensor=s4.tensor, offset=s4.offset,
                        ap=[s4.ap[0], [4, ng * M]]),
            in1=bass.AP(tensor=s4.tensor, offset=s4.offset + 1,
                        ap=[s4.ap[0], [4, ng * M]]),
            op=A.add)
        # softmax numerator: p = exp(s/sqrt8), written REPLICATED over e by
        # re-reading s 8x (step-0 input dim) -> prod2 becomes a flat 2x TT.
        # Emitted in tile-halves so ACT exp overlaps DVE prod2 in compute_b.
        p_rep = wpool.tile([128, ng, M, EH], BF, tag="prep")
        hj = ng * M // 2
        for hf in range(2):
            sl = slice(hf * hj, (hf + 1) * hj)
            nc.scalar.activation(
                out=p_rep.rearrange("p g m e -> p (g m) e")[:, sl],
                in_=bass.AP(tensor=s_sb.tensor, offset=s_sb.offset + hf * hj,
                            ap=[s_sb.ap[0], [1, hj], [0, EH]]),
                func=AF.Exp, scale=ISQ8)
        return g_v, p_rep, scr16

    def compute_b(h, state, t0=0, t1=NT):
        """Weighted sum + z + (k,m)-folds for head h, tiles [t0:t1) (DVE)."""
        g_v, p_rep, scr16 = state
        nt = t1 - t0
        ng = nt * TOPK
        j0 = t0 * TOPK * M                 # (g m)-flat element base
        base = j0 * EH                     # fully-flat element base
        prod2 = wpool.tile([128, NT * TOPK, M, EH], BF, tag="prod")
        hj = ng * M // 2
        for hf in range(2):
            sl = slice(j0 + hf * hj, j0 + (hf + 1) * hj)
            nc.vector.tensor_tensor(
                out=prod2.rearrange("p g m e -> p (g m) e")[:, sl],
                in0=g_v.rearrange("p g m e -> p (g m) e")[:, sl],
                in1=p_rep.rearrange("p g m e -> p (g m) e")[:, sl],
                op=A.mult)
        # z[p, t] = sum over (tk, m) of p (read one e-lane of the replica)
        nc.vector.tensor_reduce(
            out=zh[:, h, t0:t1],
            in_=bass.AP(tensor=p_rep.tensor, offset=p_rep.offset + base,
                        ap=[p_rep.ap[0], [TOPK * M * EH, nt], [EH, TOPK * M]]),
            axis=AX.X, op=A.add)
        # fold the two topk slots with a packed TT-add, then reduce over m by
        # halving folds on contiguous [m, e] runs (2x-eligible, vs 1x reduce)
        p2k = scr16[:, t0 * M * EH:t1 * M * EH].rearrange(
            "p (t m e) -> p t m e", m=M, e=EH)
        nc.vector.tensor_tensor(
            out=p2k,
            in0=bass.AP(tensor=prod2.tensor, offset=prod2.offset + base,
                        ap=[prod2.ap[0], [TOPK * M * EH, nt], [EH, M], [1, EH]]),
            in1=bass.AP(tensor=prod2.tensor,
                        offset=prod2.offset + base + M * EH,
                        ap=[prod2.ap[0], [TOPK * M * EH, nt], [EH, M], [1, EH]]),
            op=A.add)
        w = (M // 2) * EH
        while w > EH:
            nc.vector.tensor_tensor(
                out=bass.AP(tensor=p2k.tensor, offset=p2k.offset,
                            ap=[p2k.ap[0], [M * EH, nt], [1, w]]),
                in0=bass.AP(tensor=p2k.tensor, offset=p2k.offset,
                            ap=[p2k.ap[0], [M * EH, nt], [1, w]]),
                in1=bass.AP(tensor=p2k.tensor, offset=p2k.offset + w,
                            ap=[p2k.ap[0], [M * EH, nt], [1, w]]),
                op=A.add)
            w //= 2
        nc.vector.tensor_tensor(
            out=attn16[:, t0:t1, h, :],
            in0=bass.AP(tensor=p2k.tensor, offset=p2k.offset,
                        ap=[p2k.ap[0], [M * EH, nt], [1, EH]]),
            in1=bass.AP(tensor=p2k.tensor, offset=p2k.offset + EH,
                        ap=[p2k.ap[0], [M * EH, nt], [1, EH]]),
            op=A.add)

    # finalize tiles live in `big` so the per-half epilogue can interleave
    # with head 7's split compute (attention pools still open)
    zinv = big.tile([128, H, NT], FP)
    attn = big.tile([128, NT, D], FP)
    y = big.tile([128, NT, D], FP)
    sq2 = big.tile([128, NT, D], FP)
    ri2 = big.tile([128, NT], FP)
    ln2 = big.tile([128, NT], FP)
    x2 = big.tile([128, NT, D], F16)
    n2w_sb = consts.tile([128, D], FP)
    nc.sync.dma_start(out=n2w_sb,
                      in_=bass.AP(tensor=n2w_d, offset=0, ap=[[0, 128], [1, D]]))

    def finalize_a(t0, t1):
        """attn normalize + residual + rmsnorm2 + x2 for tiles [t0:t1)."""
        nt = t1 - t0
        nc.vector.reciprocal(out=zinv[:, :, t0:t1], in_=zh[:, :, t0:t1])
        zinv_bc = bass.AP(tensor=zinv.tensor, offset=zinv.offset + t0,
                          ap=[zinv.ap[0], [1, nt], [NT, H], [0, EH]])
        nc.vector.tensor_tensor(
            out=attn.rearrange("p t (h e) -> p t h e", e=EH)[:, t0:t1],
            in0=attn16[:, t0:t1], in1=zinv_bc, op=A.mult)
        nc.vector.tensor_tensor(out=y[:, t0:t1], in0=x_own[:, t0:t1],
                                in1=attn[:, t0:t1], op=A.add)
        nc.scalar.activation(out=sq2[:, t0:t1], in_=y[:, t0:t1], func=AF.Square)
        nc.vector.tensor_reduce(out=ri2[:, t0:t1], in_=sq2[:, t0:t1],
                                axis=AX.X, op=A.add)
        nc.vector.tensor_scalar(ri2[:, t0:t1], ri2[:, t0:t1], 1.0 / D, EPS,
                                op0=A.mult, op1=A.add)
        nc.scalar.activation(out=ln2[:, t0:t1], in_=ri2[:, t0:t1], func=AF.Ln)
        nc.scalar.activation(out=ri2[:, t0:t1], in_=ln2[:, t0:t1],
                             func=AF.Exp, scale=-0.5)
        nc.vector.tensor_tensor(out=x2[:, t0:t1], in0=y[:, t0:t1],
                                in1=_bc(ri2[:, t0:t1], 2, D), op=A.mult)
        nc.vector.tensor_tensor(out=x2[:, t0:t1], in0=x2[:, t0:t1],
                                in1=_bc(n2w_sb, 1, nt), op=A.mult)

    # Software pipeline over heads. Emission order IS execution order on each
    # strict-FIFO engine, so head h+1's selection/gather is emitted in two
    # tile-halves AROUND head h's exp: ACT runs [mT/evac(h+1) 0-3, exp(h),
    # mT/evac(h+1) 4-7] while DVE runs [masks(h+1) 0-3, prod(h), e-folds(h),
    # masks(h+1) 4-7, prod2(h), folds(h)] with no exp-wait bubble.
    half0, half1 = list(range(NT // 2)), list(range(NT // 2, NT))
    g_cur = gather_head(0, selection(0, half0 + half1), half0 + half1)
    for h in range(H - 1):
        m0 = selection(h + 1, half0)
        g_next = gather_head(h + 1, m0, half0)
        st = compute_a(h, g_cur)
        m1 = selection(h + 1, half1)
        gather_head(h + 1, m1, half1, g_sb=g_next)
        compute_b(h, st)
        g_cur = g_next
    st_last = compute_a(H - 1, g_cur)
    ps_g_cm.__exit__(None, None, None)
    ps_mt_cm.__exit__(None, None, None)
    ps_lt_cm.__exit__(None, None, None)
    sel_cm.__exit__(None, None, None)

    mlpw = ctx.enter_context(tc.tile_pool(name="mlpw", bufs=1))
    ps_m = ctx.enter_context(tc.tile_pool(name="ps_m", bufs=1, space="PSUM"))
    x2T = mlpw.tile([64, NT, 128], F16)
    hhT = mlpw.tile([128, 2, NT, 128], F16)   # [j128, c, nt, n]
    final = mlpw.tile([128, NT, D], FP)

    def mlp_half(t0, t1):
        """x2T transposes + SwiGLU MLP (fp16) + residual + out DMA for [t0:t1)."""
        nt = t1 - t0
        for t in range(t0, t1):
            xt_ps = ps_m.tile([64, 128], F16, tag="tr")
            nc.tensor.transpose(xt_ps, x2[:, t, :], identH)
            nc.scalar.copy(out=x2T[:, t, :], in_=xt_ps)
        nc.vector.tensor_tensor(out=y[:, t0:t1], in0=y[:, t0:t1],
                                in1=_bc(b3_sb, 1, nt), op=A.add)
        for c in range(2):
            h1_ps = ps_m.tile([128, nt, 128], FP, tag="h1")
            h2_ps = ps_m.tile([128, nt, 128], FP, tag="h2")
            nc.tensor.matmul(h1_ps.rearrange("j t n -> j (t n)"),
                             lhsT=w1T[:, 128 * c:128 * (c + 1)],
                             rhs=x2T[:, t0:t1].rearrange("d t n -> d (t n)"),
                             start=True, stop=True)
            nc.tensor.matmul(h2_ps.rearrange("j t n -> j (t n)"),
                             lhsT=w2T[:, 128 * c:128 * (c + 1)],
                             rhs=x2T[:, t0:t1].rearrange("d t n -> d (t n)"),
                             start=True, stop=True)
            h1b = mlpw.tile([128, nt, 128], F16, tag="h1b")
            nc.vector.tensor_scalar(h1b, h1_ps, b1_sb[:, c:c + 1], None,
                                    op0=A.add)
            sgm = mlpw.tile([128, nt, 128], F16, tag="sgm")
            nc.scalar.activation(out=sgm, in_=h1b, func=AF.Silu)
            h2s = mlpw.tile([128, nt, 128], F16, tag="h2s")
            nc.vector.tensor_scalar(h2s, h2_ps, b2_sb[:, c:c + 1], None,
                                    op0=A.add)
            nc.vector.tensor_tensor(out=hhT[:, c, t0:t1], in0=sgm, in1=h2s,
                                    op=A.mult)
        for t in range(t0, t1):
            o_ps = ps_m.tile([128, D], FP, tag="o")
            for c in range(2):
                nc.tensor.matmul(o_ps, lhsT=hhT[:, c, t, :], rhs=w3T[:, c, :],
                                 start=(c == 0), stop=(c == 1))
            nc.vector.scalar_tensor_tensor(out=final[:, t], in0=o_ps,
                                           scalar=1.0, in1=y[:, t],
                                           op0=A.mult, op1=A.add)
        nc.sync.dma_start(
            out=out_d.ap().rearrange("(t p) d -> p t d", p=128)[:, t0:t1],
            in_=final[:, t0:t1])

    # Split epilogue: head 7's second-half folds overlap finalize of the
    # first half; the MLP halves pipeline behind finalize.
    compute_b(H - 1, st_last, 0, NT // 2)
    finalize_a(0, NT // 2)
    compute_b(H - 1, st_last, NT // 2, NT)
    finalize_a(NT // 2, NT)
    mlp_half(0, NT // 2)
    mlp_half(NT // 2, NT)


def build_program():
    from contextlib import ExitStack
    nc = bacc.Bacc("TRN2", target_bir_lowering=False, debug=False,
                   num_devices=NCORES)
    x_d = nc.dram_tensor("x", [N, D], FP, kind="ExternalInput")
    pos_d = nc.dram_tensor("pos", [N, D], FP, kind="ExternalInput")
    n1w_d = nc.dram_tensor("norm1_w", [D], FP, kind="ExternalInput")
    n2w_d = nc.dram_tensor("norm2_w", [D], FP, kind="ExternalInput")
    w1_d = nc.dram_tensor("w1_w", [DH, D], FP, kind="ExternalInput")
    b1_d = nc.dram_tensor("w1_b", [DH], FP, kind="ExternalInput")
    w2_d = nc.dram_tensor("w2_w", [DH, D], FP, kind="ExternalInput")
    b2_d = nc.dram_tensor("w2_b", [DH], FP, kind="ExternalInput")
    w3_d = nc.dram_tensor("w3_w", [D, DH], FP, kind="ExternalInput")
    b3_d = nc.dram_tensor("w3_b", [D], FP, kind="ExternalInput")
    out_d = nc.dram_tensor("out", [NPC, D], FP, kind="ExternalOutput")
    xr_dram = nc.dram_tensor("xr_own", [NPC, D], FP)

    tensors = (x_d, pos_d, n1w_d, n2w_d, w1_d, b1_d, w2_d, b2_d, w3_d, b3_d,
               out_d, xr_dram)
    with tile.TileContext(nc) as tc:
        with ExitStack() as ctx:
            build_kernel_body(nc, tc, ctx, tensors)
    nc.compile()
    return nc


_NC_CACHE = None


def _get_nc():
    global _NC_CACHE
    if _NC_CACHE is None:
        _NC_CACHE = build_program()
    return _NC_CACHE


def make_in_maps(inputs):
    in_maps = []
    for c in range(NCORES):
        sh = c * NPC
        m = {
            "x": np.ascontiguousarray(np.roll(inputs["x"], -sh, axis=0)),
            "pos": np.ascontiguousarray(np.roll(inputs["pos"], -sh, axis=0)),
            "norm1_w": inputs["norm1_w"], "norm2_w": inputs["norm2_w"],
            "w1_w": inputs["w1_w"], "w1_b": inputs["w1_b"],
            "w2_w": inputs["w2_w"], "w2_b": inputs["w2_b"],
            "w3_w": inputs["w3_w"], "w3_b": inputs["w3_b"],
        }
        in_maps.append({k: np.asarray(v, dtype=np.float32) for k, v in m.items()})
    return in_maps


def run_on_hw(inputs, trace=False):
    from concourse.bass_utils import run_bass_kernel_spmd
    nc = _get_nc()
    res = run_bass_kernel_spmd(nc, make_in_maps(inputs), list(range(NCORES)),
                               trace=trace)
    out = np.concatenate([res.results[c]["out"] for c in range(NCORES)], axis=0)
    return out, res


def kernel(**inputs):
    out, _ = run_on_hw(inputs, trace=False)
    return out



# revision 63
# speedup vs baseline: 1.0375x; 1.0068x over previous
"""Trainium2 Bass kernel for nn_BasicLayer (Erwin NSA-MSA sparse ball attention).

8 NeuronCores, data-parallel over points: each core receives the full x/pos
ROTATED so its own 1024 points sit at rows [0:1024] (whole-ball rotation makes
one SPMD program correct for every core; no collectives).

Per core:
  - Stage A: xr = rmsnorm(x)*n1w + rel for all 8192 points, ball-per-partition;
    bf16 copy of xr kept as the gather source; ball-summary keys by reduction.
    Ball pos-means via flat halving folds; rsqrt = exp(-0.5*ln(v)) + Newton
    (keeps ACT in the natural_log_exp table set used by attention exp).
  - Routing logits via 4-term bf16-split PE matmuls (bit-exact vs fp32 ref, so
    top-2 ball selection matches jax.lax.top_k except true fp32 ties).
  - Per (head, tile): DVE max8 -> equality masks [n,b] in bf16 -> PE transpose
    -> PE one-hot gather matmuls -> single PSUM->SBUF bf16 evac (ACT).
  - Per head, batched across tiles (DVE tensor_reduce runs at 1x, so all
    reductions are pairwise halving folds that hit the 2x bf16/fp16 TT mode):
    scores product (2x: e-innermost broadcast AP packs) + 3 e-pair-folds;
    exp emitted e-REPLICATED by ACT (step-0 input re-read) so the
    weighted-sum product is a flat 2x TT, in tile-halves for ACT/DVE overlap;
    z by strided reduce of one e-lane; (k,m)-reduce via contiguous [m,e]-run
    halving folds in fp16.
  - Residual + RMSNorm + SwiGLU MLP (fp16 weights/transposes/matmuls on PE --
    fp32 PE runs at quarter rate; Silu in one ACT op, emitted last so its
    table set loads once).
"""

import numpy as np

import concourse.bacc as bacc
import concourse.bass as bass
import concourse.mybir as mybir
import concourse.tile as tile
from concourse.masks import make_identity

FP = mybir.dt.float32
BF = mybir.dt.bfloat16
F16 = mybir.dt.float16
U16 = mybir.dt.uint16
I16 = mybir.dt.int16

N, D = 8192, 64
M = 64          # ball size
NB = N // M     # 128 balls
H, EH = 8, 8
TOPK = 2
NCORES = 8
NPC = N // NCORES   # 1024 points per core
NT = NPC // 128     # 8 point-tiles of 128
BPC = NPC // M      # 16 own balls per core
DH = D * 4          # 256 mlp hidden
EPS = 1.1920929e-07
ISQ8 = float(1.0 / np.sqrt(EH))
EM = M * EH         # 512 = gathered elem size (m-major, e innermost)
NG = NT * TOPK      # 16 gather slots per point
NIDX = NG * 128     # 2048 gathered blocks per head

A = mybir.AluOpType
AF = mybir.ActivationFunctionType
AX = mybir.AxisListType


def _bc(ap, dim, count):
    """Insert a step-0 (broadcast) dim at position `dim` of an AP."""
    new = [list(p) for p in ap.ap]
    new.insert(dim, [0, count])
    return bass.AP(tensor=ap.tensor, offset=ap.offset, ap=new)


def build_kernel_body(nc, tc, ctx, tensors):
    (x_d, pos_d, n1w_d, n2w_d, w1_d, b1_d, w2_d, b2_d, w3_d, b3_d,
     out_d, xr_dram) = tensors

    consts = ctx.enter_context(tc.tile_pool(name="consts", bufs=1))
    big = ctx.enter_context(tc.tile_pool(name="big", bufs=1))
    front_cm = tc.tile_pool(name="front", bufs=1)
    front = front_cm.__enter__()
    ps_tr_cm = tc.tile_pool(name="ps_tr", bufs=2, space="PSUM")
    ps_tr = ps_tr_cm.__enter__()

    ident = consts.tile([128, 128], FP)
    make_identity(nc, ident)


    # ---------------- Stage A: load + xr = rmsnorm(x)*n1w + rel (ball-major) ----
    x_bm = front.tile([128, M, D], FP)       # [ball, m, d]
    pos_bm = front.tile([128, M, D], FP)
    # x and pos on different DMA queues so the two 2MB loads overlap
    nc.sync.dma_start(out=x_bm, in_=x_d.ap().rearrange("(b m) d -> b m d", m=M))
    nc.gpsimd.dma_start(out=pos_bm,
                        in_=pos_d.ap().rearrange("(b m) d -> b m d", m=M))

    n1w_sb = consts.tile([128, D], FP)
    nc.sync.dma_start(out=n1w_sb,
                      in_=bass.AP(tensor=n1w_d, offset=0, ap=[[0, 128], [1, D]]))

    # ball mean of pos (over m): flat contiguous halving folds (m-major)
    mpf = front.tile([128, 32, D], FP, tag="mpf")
    nc.vector.tensor_tensor(out=mpf, in0=pos_bm[:, 0:32, :],
                            in1=pos_bm[:, 32:64, :], op=A.add)
    w = 16
    while w >= 1:
        nc.vector.tensor_tensor(out=mpf[:, 0:w, :], in0=mpf[:, 0:w, :],
                                in1=mpf[:, w:2 * w, :], op=A.add)
        w //= 2
    mp = front.tile([128, D], FP, tag="mp")
    nc.vector.tensor_scalar(mp, mpf[:, 0, :], 1.0 / M, None, op0=A.mult)

    # rms: 1/sqrt(mean(x^2) + eps)
    sq = front.tile([128, M, D], FP, tag="sq")
    nc.scalar.activation(out=sq, in_=x_bm, func=AF.Square)
    sq8 = front.tile([128, M, 8], FP, tag="sq8")
    nc.vector.tensor_reduce(out=sq8, in_=sq.rearrange("b m (g d) -> b m g d", g=8),
                            axis=AX.X, op=A.add)
    msq = front.tile([128, M], FP, tag="msq")
    nc.vector.tensor_reduce(out=msq, in_=sq8, axis=AX.X, op=A.add)
    nc.vector.tensor_scalar(msq, msq, 1.0 / D, EPS, op0=A.mult, op1=A.add)
    rinv = front.tile([128, M], FP, tag="rinv")
    lnv = front.tile([128, M], FP, tag="lnv")
    nc.scalar.activation(out=lnv, in_=msq, func=AF.Ln)
    nc.scalar.activation(out=rinv, in_=lnv, func=AF.Exp, scale=-0.5)
    # one Newton step: r <- r*(1.5 - 0.5*msq*r^2)
    rsqv = front.tile([128, M], FP, tag="rsqv")
    nc.vector.tensor_tensor(out=rsqv, in0=rinv, in1=rinv, op=A.mult)
    nc.vector.tensor_tensor(out=rsqv, in0=rsqv, in1=msq, op=A.mult)
    nc.vector.tensor_scalar(rsqv, rsqv, -0.5, 1.5, op0=A.mult, op1=A.add)
    nc.vector.tensor_tensor(out=rinv, in0=rinv, in1=rsqv, op=A.mult)

    nc.vector.tensor_tensor(out=pos_bm, in0=pos_bm, in1=_bc(mp, 1, M),
                            op=A.subtract)      # pos_bm becomes rel
    xr_bm = front.tile([128, M, D], FP)
    nc.vector.tensor_tensor(out=xr_bm, in0=x_bm, in1=_bc(rinv, 2, D), op=A.mult)
    nc.vector.tensor_tensor(out=xr_bm, in0=xr_bm, in1=_bc(n1w_sb, 1, M), op=A.mult)
    nc.vector.tensor_tensor(out=xr_bm, in0=xr_bm, in1=pos_bm, op=A.add)

    # ---------------- Stage B: KB to DRAM (bf16) + ball-summary keys ----------
    xr16 = big.tile([128, M, D], BF)
    nc.vector.tensor_copy(out=xr16, in_=xr_bm)

    # ball-summary keys: flat halving folds over m (strided reduce is 2x slower)
    kf = front.tile([128, 32, D], FP, tag="kf")
    nc.vector.tensor_tensor(out=kf, in0=xr_bm[:, 0:32, :],
                            in1=xr_bm[:, 32:64, :], op=A.add)
    w = 16
    while w >= 1:
        nc.vector.tensor_tensor(out=kf[:, 0:w, :], in0=kf[:, 0:w, :],
                                in1=kf[:, w:2 * w, :], op=A.add)
        w //= 2
    keys_bm = kf[:, 0, :]
    keysT = front.tile([64, 128], FP)                 # [(h e), ball]
    kt_ps = ps_tr.tile([64, 128], FP, tag="tr")
    nc.tensor.transpose(kt_ps, keys_bm, ident)
    nc.scalar.copy(out=keysT, in_=kt_ps)

    # ---------------- Stage C: own-point layouts --------------------------------
    # xr rows [0:1024] -> DRAM bounce -> point-major + transposed copies
    nc.sync.dma_start(out=xr_dram.ap().rearrange("(b m) d -> b m d", m=M),
                      in_=xr_bm[0:BPC, :, :])
    q_pm = big.tile([128, NT, H, EH], FP)  # per-partition q scalars
    nc.sync.dma_start(out=q_pm,
                      in_=xr_dram.ap().rearrange("(t p) (h e) -> p t h e", p=128, e=EH))
    x_own = big.tile([128, NT, D], FP)
    nc.sync.dma_start(out=x_own,
                      in_=x_d.ap()[0:NPC, :].rearrange("(t p) d -> p t d", p=128))

    qT = front.tile([64, NT, 128], FP)       # [(h e), nt, n128]
    for t in range(NT):
        q_ps = ps_tr.tile([64, 128], FP, tag="tr")
        nc.tensor.transpose(q_ps, q_pm[:, t].rearrange("p h e -> p (h e)"), ident)
        nc.scalar.copy(out=qT[:, t, :], in_=q_ps)

    # hi/lo bf16 split: 4-term split q.k = qhi.khi + qhi.klo + qlo.khi + qlo.klo
    # -> every bf16 product exact in fp32 -> logits match fp32 ref to ~2 ulp.
    # K-row order p = e*4 + j ; k terms [khi, klo, khi, klo], q [qhi, qhi, qlo, qlo]
    identB = consts.tile([128, 128], BF)
    nc.vector.tensor_copy(out=identB, in_=ident)
    kst4 = front.tile([64, 4, 128], BF)
    qst4 = front.tile([64, 4, NT, 128], BF)
    tmp32 = front.tile([64, NT, 128], FP, tag="tmp32")
    for (src_ap, dst, nfree) in ((keysT, kst4, 1), (qT, qst4, NT)):
        nc.vector.tensor_copy(out=dst[:, 0], in_=src_ap)     # hi (cast bf16)
        t32 = tmp32[:, 0:nfree, :] if nfree == NT else tmp32[:, 0, :]
        nc.vector.tensor_copy(out=t32, in_=dst[:, 0])        # hi back to fp32
        nc.vector.tensor_tensor(out=t32, in0=src_ap, in1=t32, op=A.subtract)
        nc.vector.tensor_copy(out=dst[:, 1], in_=t32)        # lo (cast bf16)
        if nfree == 1:   # k: [khi, klo, khi, klo]
            nc.vector.tensor_copy(out=dst[:, 2], in_=dst[:, 0])
            nc.vector.tensor_copy(out=dst[:, 3], in_=dst[:, 1])
        else:            # q: [qhi, qhi, qlo, qlo]
            nc.vector.tensor_copy(out=dst[:, 2], in_=dst[:, 1])
            nc.vector.tensor_copy(out=dst[:, 3], in_=dst[:, 1])
            nc.vector.tensor_copy(out=dst[:, 1], in_=dst[:, 0])
    kstack = big.tile([32, H, 128], BF)
    qstack = big.tile([32, H, NT, 128], BF)
    for h in range(H):
        # two DMA queues so the 16 strided loads overlap (head order kept)
        nc.sync.dma_start(
            out=kstack[:, h, :],
            in_=bass.AP(tensor=kst4.tensor, offset=kst4.offset + 8 * h * 4 * 128,
                        ap=[[4 * 128, 8], [128, 4], [1, 128]]))
        nc.gpsimd.dma_start(
            out=qstack[:, h],
            in_=bass.AP(tensor=qst4.tensor,
                        offset=qst4.offset + 8 * h * 4 * NT * 128,
                        ap=[[4 * NT * 128, 8], [NT * 128, 4], [128, NT],
                            [1, 128]]))
    # weights: w1/w2 [256, 64] -> transposed fp16 [64, 256]; w3 [64, 256] ->
    # [256, 64] (fp16: fp32 PE matmuls/transposes run at quarter rate)
    w1n = consts.tile([128, 2, D], FP)
    w2n = consts.tile([128, 2, D], FP)
    w3n = consts.tile([64, DH], FP)
    nc.sync.dma_start(out=w1n, in_=w1_d.ap().rearrange("(c j) d -> j c d", j=128))
    nc.sync.dma_start(out=w2n, in_=w2_d.ap().rearrange("(c j) d -> j c d", j=128))
    nc.sync.dma_start(out=w3n, in_=w3_d.ap())
    w1h = consts.tile([128, 2, D], F16)
    w2h = consts.tile([128, 2, D], F16)
    w3h = consts.tile([64, DH], F16)
    nc.vector.tensor_copy(out=w1h, in_=w1n)
    nc.vector.tensor_copy(out=w2h, in_=w2n)
    nc.vector.tensor_copy(out=w3h, in_=w3n)
    identH = consts.tile([128, 128], F16)
    nc.vector.tensor_copy(out=identH, in_=ident)
    w1T = consts.tile([64, DH], F16)   # [d, j]
    w2T = consts.tile([64, DH], F16)
    w3T = consts.tile([128, 2, D], F16)  # [j128, c, e]
    for c in range(2):
        t_ps = ps_tr.tile([64, 128], F16, tag="trh")
        nc.tensor.transpose(t_ps, w1h[:, c, :], identH)
        nc.scalar.copy(out=w1T[:, 128 * c:128 * (c + 1)], in_=t_ps)
        t_ps2 = ps_tr.tile([64, 128], F16, tag="trh")
        nc.tensor.transpose(t_ps2, w2h[:, c, :], identH)
        nc.scalar.copy(out=w2T[:, 128 * c:128 * (c + 1)], in_=t_ps2)
        t_ps3 = ps_tr.tile([128, 64], F16, tag="trh2")
        nc.tensor.transpose(t_ps3, w3h[:, 128 * c:128 * (c + 1)],
                            identH[0:64, 0:64])
        nc.scalar.copy(out=w3T[:, c, :], in_=t_ps3)
    b1_sb = consts.tile([128, 2], FP)
    b2_sb = consts.tile([128, 2], FP)
    nc.sync.dma_start(out=b1_sb, in_=b1_d.ap().rearrange("(c j) -> j c", j=128))
    nc.sync.dma_start(out=b2_sb, in_=b2_d.ap().rearrange("(c j) -> j c", j=128))
    b3_sb = consts.tile([128, D], FP)
    nc.sync.dma_start(out=b3_sb,
                      in_=bass.AP(tensor=b3_d, offset=0, ap=[[0, 128], [1, D]]))

    front_cm.__exit__(None, None, None)
    ps_tr_cm.__exit__(None, None, None)

    # ---------------- Stage D+E: selection + PE one-hot gather + attention -----
    # Per (h, t): PE logits -> DVE max8 -> DVE equality masks [n, b] (bf16)
    # -> PE transpose -> evac maskT -> PE gather matmuls -> evac G to bf16.
    # Then per-head batched DVE scores/softmax/weighted-sum (as v2).
    gpool = ctx.enter_context(tc.tile_pool(name="gpool", bufs=4))
    wpool = ctx.enter_context(tc.tile_pool(name="wpool", bufs=2))
    sel_cm = tc.tile_pool(name="sel", bufs=2)   # quad-batched: <=2 live/tag
    sel = sel_cm.__enter__()
    ps_lt_cm = tc.tile_pool(name="ps_lt", bufs=2, space="PSUM")
    ps_lt = ps_lt_cm.__enter__()
    ps_mt_cm = tc.tile_pool(name="ps_mt", bufs=2, space="PSUM")
    ps_mt = ps_mt_cm.__enter__()
    ps_g_cm = tc.tile_pool(name="ps_g", bufs=2, space="PSUM")
    ps_g = ps_g_cm.__enter__()

    attn16 = big.tile([128, NT, H, EH], BF)
    zh = big.tile([128, H, NT], FP)

    def selection(h, ts):
        """Logits (PE) + top-2 masks (DVE) for head h, tiles ts; returns masks.

        Split from the gather so its DVE ops can be emitted while head h-1's
        ACT exp runs, and the gather's ACT ops (maskT + evac) can be emitted
        AFTER head h-1's exp — engines are strict FIFO, so emission order on
        each engine is execution order."""
        masks = {}
        nq = len(ts)
        # all of ts' logits in one PSUM tile (nq*512B <= 1 bank) -> ONE
        # is_equal for all masks (PSUM access penalty amortizes over nq*256)
        lpm2 = ps_lt.tile([128, nq, 128], FP, tag="lt")
        for j, t in enumerate(ts):
            nc.tensor.matmul(lpm2[:, j], lhsT=qstack[:, h, t, :],
                             rhs=kstack[:, h, :], start=True, stop=True)
        v8p = sel.tile([128, nq, 8], FP, tag="v8")
        for j in range(nq):
            nc.vector.max(out=v8p[:, j], in_=lpm2[:, j])
        # mask4[p, j, tk, b] = (lpm == v8[tk])
        mask4 = sel.tile([128, nq, TOPK, 128], BF, tag="mnb")
        nc.vector.tensor_tensor(
            out=mask4,
            in0=bass.AP(tensor=lpm2.tensor, offset=lpm2.offset,
                        ap=[lpm2.ap[0], [128, nq], [0, TOPK], [1, 128]]),
            in1=bass.AP(tensor=v8p.tensor, offset=v8p.offset,
                        ap=[v8p.ap[0], [8, nq], [1, TOPK], [0, 128]]),
            op=A.is_equal)
        for j, t in enumerate(ts):
            masks[t] = mask4[:, j]
        return masks

    def gather_head(h, masks, ts, g_sb=None):
        """PE transposes + one-hot gathers + ACT evacs for head h, tiles ts.

        maskT(t+1) is emitted BEFORE evac(t) so the strict-FIFO ACT queue
        doesn't serialize ACT->PE->ACT per tile: gather(t) runs on PE while
        ACT does maskT(t+1), and evac(t) follows immediately."""
        if g_sb is None:
            g_sb = gpool.tile([128, NT, TOPK, M, EH], BF, tag="g")

        # all of ts' mask transposes into ONE PSUM tile -> one ACT evac
        # (the 352-cyc ACT op overhead amortizes over nq*256 elems)
        nq = len(ts)
        mt_ps = ps_mt.tile([128, nq, TOPK, 128], BF, tag="mt")
        for j, t in enumerate(ts):
            for tk in range(TOPK):
                nc.tensor.transpose(mt_ps[:, j, tk, :],
                                    masks[t][:, tk, :], identB)
        maskT4 = sel.tile([128, nq, TOPK, 128], BF, tag="mT")
        nc.scalar.copy(out=maskT4, in_=mt_ps)

        def emit_gather(j):
            g_ps = ps_g.tile([128, TOPK, M, EH], FP, tag="g")
            for tk in range(TOPK):
                # rhs: xr16 [b, m, (h e)] -> per-head (m, e) column order
                nc.tensor.matmul(
                    g_ps[:, tk].rearrange("p m e -> p (m e)"),
                    lhsT=maskT4[:, j, tk, :],
                    rhs=bass.AP(tensor=xr16.tensor,
                                offset=xr16.offset + EH * h,
                                ap=[xr16.ap[0], [D, M], [1, EH]]),
                    start=True, stop=True)
            return g_ps

        # gather(j+1) is emitted before evac(j): PE fills while ACT drains,
        # and ps_g's 2 bufs hold exactly the in-flight pair
        gp_prev = emit_gather(0)
        for j in range(1, nq):
            gp = emit_gather(j)
            nc.scalar.copy(out=g_sb[:, ts[j - 1]], in_=gp_prev)
            gp_prev = gp
        nc.scalar.copy(out=g_sb[:, ts[nq - 1]], in_=gp_prev)
        return g_sb

    def compute_a(h, g_sb):
        """Scores + batched exp for head h (DVE prod/e-folds, ACT exp)."""
        nt = NT
        ng = nt * TOPK
        # bf16 q for this head, replicated over topk: [p, (t k), e]
        q2h = wpool.tile([128, nt, TOPK, EH], BF, tag="q2")
        nc.vector.tensor_copy(
            out=q2h,
            in_=bass.AP(tensor=q_pm.tensor,
                        offset=q_pm.offset + EH * h,
                        ap=[q_pm.ap[0], [H * EH, nt], [0, TOPK], [1, EH]]))
        g_v = g_sb.rearrange("p t k m e -> p (t k) m e")
        prod = wpool.tile([128, ng, M, EH], BF, tag="prod")
        q2_bc = bass.AP(tensor=q2h.tensor, offset=q2h.offset,
                        ap=[q2h.ap[0], [EH, ng], [0, M], [1, EH]])
        nc.vector.tensor_tensor(out=prod, in0=g_v, in1=q2_bc, op=A.mult)
        # s[p, (g m)] fp16 = sum_e prod, via 3 pair-folds over e (e pairs are
        # step-1 runs of 4/2/1 -> cheaper than the 1x-mode strided reduce).
        # scr16 is shared by s4 (e-folds) and p2k (m-folds) — disjoint lifetimes
        scr16 = wpool.tile([128, NT * M * EH], F16, tag="scr")
        s4 = scr16[:, 0:ng * M * 4].rearrange("p (j f) -> p j f", f=4)
        nc.vector.tensor_tensor(
            out=s4,
            in0=bass.AP(tensor=prod.tensor, offset=prod.offset,
                        ap=[prod.ap[0], [EH, ng * M], [1, 4]]),
            in1=bass.AP(tensor=prod.tensor, offset=prod.offset + 4,
                        ap=[prod.ap[0], [EH, ng * M], [1, 4]]),
            op=A.add)
        nc.vector.tensor_tensor(
            out=bass.AP(tensor=s4.tensor, offset=s4.offset,
                        ap=[s4.ap[0], [4, ng * M], [1, 2]]),
            in0=bass.AP(tensor=s4.tensor, offset=s4.offset,
                        ap=[s4.ap[0], [4, ng * M], [1, 2]]),
            in1=bass.AP(tensor=s4.tensor, offset=s4.offset + 2,
                        ap=[s4.ap[0], [4, ng * M], [1, 2]]),
            op=A.add)
        s_sb = wpool.tile([128, ng, M], F16, tag="s")
        nc.vector.tensor_tensor(
            out=s_sb,
            in0=bass.AP(tensor=s4.tensor, offset=s4.offset,
                        ap=[s4.ap[0], [4, ng * M]]),
            in1=bass.AP(tensor=s4.tensor, offset=s4.offset + 1,
                        ap=[s4.ap[0], [4, ng * M]]),
            op=A.add)
        # softmax numerator: p = exp(s/sqrt8), written REPLICATED over e by
        # re-reading s 8x (step-0 input dim) -> prod2 becomes a flat 2x TT.
        # Emitted in tile-halves so ACT exp overlaps DVE prod2 in compute_b.
        p_rep = wpool.tile([128, ng, M, EH], BF, tag="prep")
        hj = ng * M // 2
        for hf in range(2):
            sl = slice(hf * hj, (hf + 1) * hj)
            nc.scalar.activation(
                out=p_rep.rearrange("p g m e -> p (g m) e")[:, sl],
                in_=bass.AP(tensor=s_sb.tensor, offset=s_sb.offset + hf * hj,
                            ap=[s_sb.ap[0], [1, hj], [0, EH]]),
                func=AF.Exp, scale=ISQ8)
        return g_v, p_rep, scr16

    def compute_b(h, state, t0=0, t1=NT):
        """Weighted sum + z + (k,m)-folds for head h, tiles [t0:t1) (DVE)."""
        g_v, p_rep, scr16 = state
        nt = t1 - t0
        ng = nt * TOPK
        j0 = t0 * TOPK * M                 # (g m)-flat element base
        base = j0 * EH                     # fully-flat element base
        prod2 = wpool.tile([128, NT * TOPK, M, EH], BF, tag="prod")
        hj = ng * M // 2
        for hf in range(2):
            sl = slice(j0 + hf * hj, j0 + (hf + 1) * hj)
            nc.vector.tensor_tensor(
                out=prod2.rearrange("p g m e -> p (g m) e")[:, sl],
                in0=g_v.rearrange("p g m e -> p (g m) e")[:, sl],
                in1=p_rep.rearrange("p g m e -> p (g m) e")[:, sl],
                op=A.mult)
        # z[p, t] = sum over (tk, m) of p (read one e-lane of the replica)
        nc.vector.tensor_reduce(
            out=zh[:, h, t0:t1],
            in_=bass.AP(tensor=p_rep.tensor, offset=p_rep.offset + base,
                        ap=[p_rep.ap[0], [TOPK * M * EH, nt], [EH, TOPK * M]]),
            axis=AX.X, op=A.add)
        # fold the two topk slots with a packed TT-add, then reduce over m by
        # halving folds on contiguous [m, e] runs (2x-eligible, vs 1x reduce)
        p2k = scr16[:, t0 * M * EH:t1 * M * EH].rearrange(
            "p (t m e) -> p t m e", m=M, e=EH)
        nc.vector.tensor_tensor(
            out=p2k,
            in0=bass.AP(tensor=prod2.tensor, offset=prod2.offset + base,
                        ap=[prod2.ap[0], [TOPK * M * EH, nt], [EH, M], [1, EH]]),
            in1=bass.AP(tensor=prod2.tensor,
                        offset=prod2.offset + base + M * EH,
                        ap=[prod2.ap[0], [TOPK * M * EH, nt], [EH, M], [1, EH]]),
            op=A.add)
        w = (M // 2) * EH
        while w > EH:
            nc.vector.tensor_tensor(
                out=bass.AP(tensor=p2k.tensor, offset=p2k.offset,
                            ap=[p2k.ap[0], [M * EH, nt], [1, w]]),
                in0=bass.AP(tensor=p2k.tensor, offset=p2k.offset,
                            ap=[p2k.ap[0], [M * EH, nt], [1, w]]),
                in1=bass.AP(tensor=p2k.tensor, offset=p2k.offset + w,
                            ap=[p2k.ap[0], [M * EH, nt], [1, w]]),
                op=A.add)
            w //= 2
        nc.vector.tensor_tensor(
            out=attn16[:, t0:t1, h, :],
            in0=bass.AP(tensor=p2k.tensor, offset=p2k.offset,
                        ap=[p2k.ap[0], [M * EH, nt], [1, EH]]),
            in1=bass.AP(tensor=p2k.tensor, offset=p2k.offset + EH,
                        ap=[p2k.ap[0], [M * EH, nt], [1, EH]]),
            op=A.add)

    # finalize tiles live in `big` so the per-half epilogue can interleave
    # with head 7's split compute (attention pools still open)
    zinv = big.tile([128, H, NT], FP)
    attn = big.tile([128, NT, D], FP)
    y = big.tile([128, NT, D], FP)
    sq2 = big.tile([128, NT, D], FP)
    ri2 = big.tile([128, NT], FP)
    ln2 = big.tile([128, NT], FP)
    x2 = big.tile([128, NT, D], F16)
    n2w_sb = consts.tile([128, D], FP)
    nc.sync.dma_start(out=n2w_sb,
                      in_=bass.AP(tensor=n2w_d, offset=0, ap=[[0, 128], [1, D]]))

    def finalize_a(t0, t1):
        """attn normalize + residual + rmsnorm2 + x2 for tiles [t0:t1)."""
        nt = t1 - t0
        nc.vector.reciprocal(out=zinv[:, :, t0:t1], in_=zh[:, :, t0:t1])
        zinv_bc = bass.AP(tensor=zinv.tensor, offset=zinv.offset + t0,
                          ap=[zinv.ap[0], [1, nt], [NT, H], [0, EH]])
        nc.vector.tensor_tensor(
            out=attn.rearrange("p t (h e) -> p t h e", e=EH)[:, t0:t1],
            in0=attn16[:, t0:t1], in1=zinv_bc, op=A.mult)
        nc.vector.tensor_tensor(out=y[:, t0:t1], in0=x_own[:, t0:t1],
                                in1=attn[:, t0:t1], op=A.add)
        nc.scalar.activation(out=sq2[:, t0:t1], in_=y[:, t0:t1], func=AF.Square)
        nc.vector.tensor_reduce(out=ri2[:, t0:t1], in_=sq2[:, t0:t1],
                                axis=AX.X, op=A.add)
        nc.vector.tensor_scalar(ri2[:, t0:t1], ri2[:, t0:t1], 1.0 / D, EPS,
                                op0=A.mult, op1=A.add)
        nc.scalar.activation(out=ln2[:, t0:t1], in_=ri2[:, t0:t1], func=AF.Ln)
        nc.scalar.activation(out=ri2[:, t0:t1], in_=ln2[:, t0:t1],
                             func=AF.Exp, scale=-0.5)
        nc.vector.tensor_tensor(out=x2[:, t0:t1], in0=y[:, t0:t1],
                                in1=_bc(ri2[:, t0:t1], 2, D), op=A.mult)
        nc.vector.tensor_tensor(out=x2[:, t0:t1], in0=x2[:, t0:t1],
                                in1=_bc(n2w_sb, 1, nt), op=A.mult)

    # Software pipeline over heads. Emission order IS execution order on each
    # strict-FIFO engine, so head h+1's selection/gather is emitted in two
    # tile-halves AROUND head h's exp: ACT runs [mT/evac(h+1) 0-3, exp(h),
    # mT/evac(h+1) 4-7] while DVE runs [masks(h+1) 0-3, prod(h), e-folds(h),
    # masks(h+1) 4-7, prod2(h), folds(h)] with no exp-wait bubble.
    half0, half1 = list(range(NT // 2)), list(range(NT // 2, NT))
    g_cur = gather_head(0, selection(0, half0), half0)
    gather_head(0, selection(0, half1), half1, g_sb=g_cur)
    for h in range(H - 1):
        m0 = selection(h + 1, half0)
        g_next = gather_head(h + 1, m0, half0)
        st = compute_a(h, g_cur)
        m1 = selection(h + 1, half1)
        gather_head(h + 1, m1, half1, g_sb=g_next)
        compute_b(h, st)
        g_cur = g_next
    st_last = compute_a(H - 1, g_cur)
    ps_g_cm.__exit__(None, None, None)
    ps_mt_cm.__exit__(None, None, None)
    ps_lt_cm.__exit__(None, None, None)
    sel_cm.__exit__(None, None, None)

    mlpw = ctx.enter_context(tc.tile_pool(name="mlpw", bufs=1))
    ps_m = ctx.enter_context(tc.tile_pool(name="ps_m", bufs=1, space="PSUM"))
    x2T = mlpw.tile([64, NT, 128], F16)
    hhT = mlpw.tile([128, 2, NT, 128], F16)   # [j128, c, nt, n]
    final = mlpw.tile([128, NT, D], FP)

    def mlp_half(t0, t1):
        """x2T transposes + SwiGLU MLP (fp16) + residual + out DMA for [t0:t1)."""
        nt = t1 - t0
        for t in range(t0, t1):
            xt_ps = ps_m.tile([64, 128], F16, tag="tr")
            nc.tensor.transpose(xt_ps, x2[:, t, :], identH)
            nc.scalar.copy(out=x2T[:, t, :], in_=xt_ps)
        nc.vector.tensor_tensor(out=y[:, t0:t1], in0=y[:, t0:t1],
                                in1=_bc(b3_sb, 1, nt), op=A.add)
        for c in range(2):
            h1_ps = ps_m.tile([128, nt, 128], FP, tag="h1")
            h2_ps = ps_m.tile([128, nt, 128], FP, tag="h2")
            nc.tensor.matmul(h1_ps.rearrange("j t n -> j (t n)"),
                             lhsT=w1T[:, 128 * c:128 * (c + 1)],
                             rhs=x2T[:, t0:t1].rearrange("d t n -> d (t n)"),
                             start=True, stop=True)
            nc.tensor.matmul(h2_ps.rearrange("j t n -> j (t n)"),
                             lhsT=w2T[:, 128 * c:128 * (c + 1)],
                             rhs=x2T[:, t0:t1].rearrange("d t n -> d (t n)"),
                             start=True, stop=True)
            h1b = mlpw.tile([128, nt, 128], F16, tag="h1b")
            nc.vector.tensor_scalar(h1b, h1_ps, b1_sb[:, c:c + 1], None,
                                    op0=A.add)
            sgm = mlpw.tile([128, nt, 128], F16, tag="sgm")
            nc.scalar.activation(out=sgm, in_=h1b, func=AF.Silu)
            h2s = mlpw.tile([128, nt, 128], F16, tag="h2s")
            nc.vector.tensor_scalar(h2s, h2_ps, b2_sb[:, c:c + 1], None,
                                    op0=A.add)
            nc.vector.tensor_tensor(out=hhT[:, c, t0:t1], in0=sgm, in1=h2s,
                                    op=A.mult)
        for t in range(t0, t1):
            o_ps = ps_m.tile([128, D], FP, tag="o")
            for c in range(2):
                nc.tensor.matmul(o_ps, lhsT=hhT[:, c, t, :], rhs=w3T[:, c, :],
                                 start=(c == 0), stop=(c == 1))
            nc.vector.scalar_tensor_tensor(out=final[:, t], in0=o_ps,
                                           scalar=1.0, in1=y[:, t],
                                           op0=A.mult, op1=A.add)
        nc.sync.dma_start(
            out=out_d.ap().rearrange("(t p) d -> p t d", p=128)[:, t0:t1],
            in_=final[:, t0:t1])

    # Split epilogue: head 7's second-half folds overlap finalize of the
    # first half; the MLP halves pipeline behind finalize.
    compute_b(H - 1, st_last, 0, NT // 2)
    finalize_a(0, NT // 2)
    compute_b(H - 1, st_last, NT // 2, NT)
    finalize_a(NT // 2, NT)
    mlp_half(0, NT // 2)
    mlp_half(NT // 2, NT)


def build_program():
    from contextlib import ExitStack
    nc = bacc.Bacc("TRN2", target_bir_lowering=False, debug=False,
                   num_devices=NCORES)
    x_d = nc.dram_tensor("x", [N, D], FP, kind="ExternalInput")
    pos_d = nc.dram_tensor("pos", [N, D], FP, kind="ExternalInput")
    n1w_d = nc.dram_tensor("norm1_w", [D], FP, kind="ExternalInput")
    n2w_d = nc.dram_tensor("norm2_w", [D], FP, kind="ExternalInput")
    w1_d = nc.dram_tensor("w1_w", [DH, D], FP, kind="ExternalInput")
    b1_d = nc.dram_tensor("w1_b", [DH], FP, kind="ExternalInput")
    w2_d = nc.dram_tensor("w2_w", [DH, D], FP, kind="ExternalInput")
    b2_d = nc.dram_tensor("w2_b", [DH], FP, kind="ExternalInput")
    w3_d = nc.dram_tensor("w3_w", [D, DH], FP, kind="ExternalInput")
    b3_d = nc.dram_tensor("w3_b", [D], FP, kind="ExternalInput")
    out_d = nc.dram_tensor("out", [NPC, D], FP, kind="ExternalOutput")
    xr_dram = nc.dram_tensor("xr_own", [NPC, D], FP)

    tensors = (x_d, pos_d, n1w_d, n2w_d, w1_d, b1_d, w2_d, b2_d, w3_d, b3_d,
               out_d, xr_dram)
    with tile.TileContext(nc) as tc:
        with ExitStack() as ctx:
            build_kernel_body(nc, tc, ctx, tensors)
    nc.compile()
    return nc


_NC_CACHE = None


def _get_nc():
    global _NC_CACHE
    if _NC_CACHE is None:
        _NC_CACHE = build_program()
    return _NC_CACHE


def make_in_maps(inputs):
    in_maps = []
    for c in range(NCORES):
        sh = c * NPC
        m = {
            "x": np.ascontiguousarray(np.roll(inputs["x"], -sh, axis=0)),
            "pos": np.ascontiguousarray(np.roll(inputs["pos"], -sh, axis=0)),
            "norm1_w": inputs["norm1_w"], "norm2_w": inputs["norm2_w"],
            "w1_w": inputs["w1_w"], "w1_b": inputs["w1_b"],
            "w2_w": inputs["w2_w"], "w2_b": inputs["w2_b"],
            "w3_w": inputs["w3_w"], "w3_b": inputs["w3_b"],
        }
        in_maps.append({k: np.asarray(v, dtype=np.float32) for k, v in m.items()})
    return in_maps


def run_on_hw(inputs, trace=False):
    from concourse.bass_utils import run_bass_kernel_spmd
    nc = _get_nc()
    res = run_bass_kernel_spmd(nc, make_in_maps(inputs), list(range(NCORES)),
                               trace=trace)
    out = np.concatenate([res.results[c]["out"] for c in range(NCORES)], axis=0)
    return out, res


def kernel(**inputs):
    out, _ = run_on_hw(inputs, trace=False)
    return out

